# revision 22
# baseline (speedup 1.0000x reference)
"""COPNLL loss kernel for Trainium2 (8 NeuronCores) — v2 (slice scheme).

Math: the reference builds V = (sig2e*I + sig2bs0*Z0 Z0^T + sig2bs1*Z1 Z1^T)/sig2
with Z0 (4096x1000), Z1 (4096x500) one-hot, then needs logdet(V) and m^T V^-1 m.
Both reduce via Woodbury to the 1500x1500 capacitance matrix whose (0,0) block
is diagonal, leaving one dense 500x500 Schur complement
    S = (sig2e/s1*I + diag(c1)) - C^T diag(1/A) C,   A = sig2e/s0 + c0
with C = Z0^T Z1 (co-occurrence counts), c0/c1 level counts, a = Z0^T m, b = Z1^T m:
    logdet(sig2*V) = (N-q)log sig2e + q0 log s0 + q1 log s1 + sum(log A) + logdet S
    m^T V^-1 m     = (sig2/sig2e) * (m^T m - a^T A^-1 a - t^T S^-1 t),
                     t = b - C^T (a/A)

Device plan (SPMD, 8 cores, FULL inputs replicated to every core):
  phase A: core p owns the 126-level window [126p, 126p+126) of the level-0
    axis. Per 128-row chunk (32 chunks = all N rows) ONE matmul with the
    stationary matrix [onehot0_window | 1 | m] (128 cols) against
    [onehot1 | 1 | m] (502 cols) accumulates, over all rows:
      rows 0..125: C slice | counts0 slice | a slice
      rows 126/127: the full G1 = [1|m]^T[Z1|1|m]  (identical on all cores)
    so counts0/a/G1/C all come from one PSUM bank. m, m^Tm, r^Tr are local.
  comm: ONE int8 AllGather (64KB/core): cols 0..500 = int8 C+counts slice,
    cols 504..507 = f32 a slice bit-cast (AllGather concatenates, bit-exact).
    A garbage warmup AllGather issued at t=0 pre-pays the NRT rendezvous
    barrier concurrently with phase A.
  phase C (redundant on all cores): S assembled tile-by-tile from the gathered
    buffer (the t-vector rides as rhs column 500); block LDL with 128-blocks;
    block inverses via Hotelling/Newton-Schulz; block logdets via Chebyshev
    trace of log accumulated on the PE.
"""

import math
import sys
import types

import numpy as np

import concourse.bass as bass
import concourse.bacc as bacc
import concourse.mybir as mybir
from concourse.bass import ds, ts
from concourse.bass_utils import run_bass_kernel_spmd
from concourse.masks import make_identity
from concourse.tile import TileContext


def _ensure_axon_hooks():
    """bass_utils imports antenv.axon_hooks when tracing; this image's antenv
    lacks it. Provide a shim (with the real ctypes NTFF hook when available)
    so trace=True/BASS_TRACE never crashes the kernel."""
    try:
        import antenv.axon_hooks  # noqa: F401
        return
    except ImportError:
        pass
    try:
        import trn_agent_boot.trn_boot as tb
        hook = tb._ntff_profile_via_ctypes("/opt/axon/libaxon_pjrt.so")
    except Exception:
        hook = None
    mod = types.ModuleType("antenv.axon_hooks")
    mod._hook = hook
    mod.get_axon_ntff_profile_hook = lambda: mod._hook

    def _set(h):
        mod._hook = h

    mod.set_axon_ntff_profile_hook = _set
    sys.modules["antenv.axon_hooks"] = mod
    try:
        import antenv
        antenv.axon_hooks = mod
    except ImportError:
        pass
    try:
        import concourse.bass_utils as bu
        _orig_upload = bu.upload_artifacts

        def _safe_upload(tmpdir):
            try:
                return _orig_upload(tmpdir)
            except Exception:
                return f"local:{tmpdir}"

        bu.upload_artifacts = _safe_upload
    except Exception:
        pass


_ensure_axon_hooks()

N = 4096
NCORES = 8
NCH = N // 128             # 32 row chunks, every core sees all of them
WIN = 126                  # level-0 window width per core (8*126=1008 >= 1000)
Q0 = 1000
Q1 = 500
FR = Q1 + 2                # rhs width: [Z1 | 1 | m]
AGW = 512                  # allgather row width (int8)
SP = 512                   # padded S size
NBLK = SP // 128           # 4
W3 = Q1 - 3 * 128          # 116: valid width of the last S block
LO, HI = 1.4, 18.0         # eigenvalue bounds for NS init + Chebyshev interval
NS_ITERS = 3
CHEB_DEG = 6
NCOEF = CHEB_DEG + 1
CLIP = 4.2648907939226017  # sqrt(2)*erfinv(1-2e-5)

F32 = mybir.dt.float32
BF16 = mybir.dt.bfloat16
I8 = mybir.dt.int8
I32 = mybir.dt.int32
U32 = mybir.dt.uint32
AX = mybir.AxisListType
OP = mybir.AluOpType
ACT = mybir.ActivationFunctionType


def cheb_coeffs(lo=LO, hi=HI, deg=CHEB_DEG):
    K = 4000
    th = (np.arange(K) + 0.5) * np.pi / K
    xk = np.cos(th)
    fk = np.log((hi - lo) / 2.0 * xk + (hi + lo) / 2.0)
    cs = np.array([2.0 / K * np.sum(fk * np.cos(j * th)) for j in range(deg + 1)])
    cs[0] *= 0.5
    return cs.astype(np.float32)


def _diag_fill(nc, tile_ap, value):
    nc.gpsimd.memset(tile_ap, 0.0)
    nc.gpsimd.affine_select(out=tile_ap, in_=tile_ap, compare_op=OP.not_equal,
                            fill=value, base=0, pattern=[[-1, 128]],
                            channel_multiplier=1)


def build_module(n_cores=NCORES):
    nc = bacc.Bacc(num_devices=n_cores)
    pk_d = nc.declare_dram_parameter("packed", [128, 4 * NCH], F32,
                                     isOutput=False)
    cst_d = nc.declare_dram_parameter("consts", [16], F32, isOutput=False)
    chb_d = nc.declare_dram_parameter("chebc", [2 * NCOEF], F32, isOutput=False)
    out_d = nc.declare_dram_parameter("out", [1, 1], F32, isOutput=True)

    agb_in = nc.dram_tensor("agb_in", [128 * AGW], I8)
    agb_out = nc.dram_tensor("agb_out", [n_cores * 128 * AGW], I8,
                             addr_space="Shared")
    warm_in = nc.dram_tensor("warm_in", [16], F32)
    warm_out = nc.dram_tensor("warm_out", [16 * n_cores], F32,
                              addr_space="Shared")

    with TileContext(nc) as tc, \
         tc.tile_pool(name="consts", bufs=1) as consts, \
         tc.tile_pool(name="work", bufs=1) as work:

        # warmup collective: pre-pays the one-time cross-core rendezvous
        # barrier concurrently with phase A. Data is garbage; result unused.
        if n_cores > 1:
            nc.gpsimd.collective_compute(
                "AllGather", OP.bypass,
                replica_groups=[list(range(n_cores))],
                ins=[warm_in[:]], outs=[warm_out[:]],
            )

        # ---- constants ----
        ident = consts.tile([128, 128], F32, tag="ident")
        make_identity(nc, ident)
        i2 = consts.tile([128, 128], F32, tag="i2")              # 2*I
        _diag_fill(nc, i2, 2.0)
        alphaI = consts.tile([128, 128], F32, tag="alphaI")      # NS init
        _diag_fill(nc, alphaI, 2.0 / (LO + HI))
        shiftI = consts.tile([128, 128], F32, tag="shiftI")      # Chebyshev shift
        _diag_fill(nc, shiftI, (HI + LO) / (HI - LO))
        ones512 = consts.tile([128, SP], F32, tag="ones512")
        nc.vector.memset(ones512, 1.0)
        identB16 = consts.tile([128, 128], BF16, tag="identB16")
        nc.vector.tensor_copy(identB16, ident)

        cst_row = consts.tile([1, 16], F32, tag="cst_row")
        nc.sync.dma_start(cst_row, cst_d[:].rearrange("(p x) -> p x", p=1))
        cst_row2 = consts.tile([1, 16], F32, tag="cst_row2")
        nc.vector.tensor_copy(cst_row2, cst_row)
        chb = consts.tile([1, 2 * NCOEF], F32, tag="chb")
        nc.sync.dma_start(chb, chb_d[:].rearrange("(p x) -> p x", p=1))
        chb2 = consts.tile([1, 2 * NCOEF], F32, tag="chb2")
        nc.vector.tensor_copy(chb2, chb)
        cst = consts.tile([128, 16], F32, tag="cst")
        chbB = consts.tile([128, 2 * NCOEF], F32, tag="chbB")
        with tc.tile_pool(name="setup_ps", bufs=2,
                          space=bass.MemorySpace.PSUM) as gps0:
            ps_b = gps0.tile([128, 16], F32, tag="gps0")
            nc.tensor.matmul(ps_b, ones512[0:1, 0:128], cst_row2,
                             start=True, stop=True)
            nc.vector.tensor_copy(cst, ps_b)
            ps_c = gps0.tile([128, 2 * NCOEF], F32, tag="gps0")
            nc.tensor.matmul(ps_c, ones512[0:1, 0:128], chb2,
                             start=True, stop=True)
            nc.vector.tensor_copy(chbB, ps_c)

        # iotas for the one-hot compares
        iota1i = work.tile([128, Q1], I32, tag="iota1i")
        nc.gpsimd.iota(iota1i, pattern=[[1, Q1]], base=0, channel_multiplier=0)
        iota1 = work.tile([128, Q1], F32, tag="iota1")
        nc.vector.tensor_copy(iota1, iota1i)
        iotaWi = work.tile([128, WIN], I32, tag="iotaWi")
        nc.gpsimd.iota(iotaWi, pattern=[[1, WIN]], base=0, channel_multiplier=0)
        iotaW = work.tile([128, WIN], F32, tag="iotaW")
        nc.vector.tensor_copy(iotaW, iotaWi)
        # shift by this core's window base (per-core const c[9])
        nc.vector.tensor_scalar(out=iotaW, in0=iotaW, scalar1=cst[:, 9:10],
                                scalar2=None, op0=OP.add)

        # ---- inputs -> m, sum r^2, sum m^2 ----
        packed = work.tile([128, 4 * NCH], F32, tag="packed")
        nc.sync.dma_start(packed, pk_d[:])
        yt = packed[:, 0:NCH]
        yp = packed[:, NCH:2 * NCH]
        idx0 = work.tile([128, NCH], F32, tag="idx0")
        nc.vector.tensor_copy(idx0, packed[:, 2 * NCH:3 * NCH].bitcast(I32))
        idx1 = work.tile([128, NCH], F32, tag="idx1")
        nc.vector.tensor_copy(idx1, packed[:, 3 * NCH:4 * NCH].bitcast(I32))
        resid = work.tile([128, NCH], F32, tag="resid")
        nc.vector.tensor_sub(resid, yt, yp)
        mvec = work.tile([128, NCH], F32, tag="mvec")
        nc.vector.tensor_scalar(out=mvec, in0=resid, scalar1=cst[:, 0:1],
                                scalar2=cst[:, 1:2], op0=OP.mult, op1=OP.min)
        nc.vector.tensor_scalar(out=mvec, in0=mvec, scalar1=cst[:, 8:9],
                                scalar2=None, op0=OP.max)
        scr_n = work.tile([128, NCH], F32, tag="scr_n")
        scal2 = work.tile([128, 2], F32, tag="scal2")
        nc.vector.tensor_mul(scr_n, resid, resid)
        nc.vector.tensor_reduce(scal2[:, 0:1], scr_n, AX.X, OP.add)
        nc.vector.tensor_mul(scr_n, mvec, mvec)
        nc.vector.tensor_reduce(scal2[:, 1:2], scr_n, AX.X, OP.add)
        scal2r = work.tile([1, 2], F32, tag="scal2r")
        with tc.tile_pool(name="sc_ps", bufs=1,
                          space=bass.MemorySpace.PSUM) as gpsc:
            ps_s = gpsc.tile([128, 2], F32, tag="gpsc")
            nc.tensor.matmul(ps_s[0:1, 0:2], ones512[:, 0:1], scal2,
                             start=True, stop=True)
            nc.vector.tensor_copy(scal2r, ps_s[0:1, 0:2])

        # ---- phase A: one matmul per 128-row chunk over all N rows ----
        G1 = work.tile([2, FR], F32, tag="G1")
        stg = work.tile([128, AGW], I8, tag="stg")
        with (
            tc.tile_pool(name="phA", bufs=3) as pha,
            tc.tile_pool(name="phA_ps", bufs=1, space=bass.MemorySpace.PSUM) as pps,
        ):
            psC = pps.tile([128, FR], F32, tag="psC")
            for c in range(NCH):
                cc = c % 3
                # cols 0/1 = [1|m] (G1 lands at psC rows 0/1: PSUM partition
                # reads must start at 0); cols 2..127 = level-window one-hot
                oh = pha.tile([128, 128], BF16, tag=f"oh_{cc}", name=f"oh_{cc}")
                nc.vector.memset(oh[:, 0:1], 1.0)
                nc.scalar.copy(oh[:, 1:2], mvec[:, c:c + 1])
                nc.vector.tensor_scalar(out=oh[:, 2:128], in0=iotaW,
                                        scalar1=idx0[:, c:c + 1],
                                        scalar2=None, op0=OP.is_equal)
                rh = pha.tile([128, FR], BF16, tag=f"rh_{cc}", name=f"rh_{cc}")
                nc.vector.tensor_scalar(out=rh[:, 0:Q1], in0=iota1,
                                        scalar1=idx1[:, c:c + 1],
                                        scalar2=None, op0=OP.is_equal)
                nc.vector.memset(rh[:, Q1:Q1 + 1], 1.0)
                nc.scalar.copy(rh[:, Q1 + 1:FR], mvec[:, c:c + 1])
                nc.tensor.matmul(psC, oh, rh, start=(c == 0),
                                 stop=(c == NCH - 1))
            # readout: G1 (rows 0/1), int8 staging for the AllGather
            nc.vector.tensor_copy(G1, psC[0:2, :])
            nc.vector.memset(stg[:, Q1 + 1:AGW], 0)
            nc.vector.tensor_copy(stg[:, 0:Q1 + 1], psC[:, 0:Q1 + 1])
            af32 = work.tile([128, 1], F32, tag="af32")
            nc.vector.tensor_copy(af32, psC[:, Q1 + 1:FR])
            nc.vector.tensor_copy(stg[:, 504:508], af32[:, 0:1].bitcast(I8))

        nc.sync.dma_start(
            agb_in[:].rearrange("(p f) -> p f", p=128), stg)
        if n_cores > 1:
            nc.gpsimd.collective_compute(
                "AllGather", OP.bypass,
                replica_groups=[list(range(n_cores))],
                ins=[agb_in[:]], outs=[agb_out[:]],
            )
        else:
            nc.sync.dma_start(agb_out[:], agb_in[:])

        # ---- phase C constants built while the collective is in flight ----
        iotaLi = work.tile([128, NCORES], I32, tag="iotaLi")
        nc.gpsimd.iota(iotaLi, pattern=[[0, NCORES]], base=0,
                       channel_multiplier=1)
        iotaL = work.tile([128, NCORES], F32, tag="iotaL")
        nc.vector.tensor_copy(iotaL, iotaLi)
        # per-tile valid rows: 2 <= row, row <= thr (level = 126t + row - 2;
        # rows 0/1 are G1 junk; tile 7 rows >= 120 are levels >= 1000 pad)
        thrM = work.tile([128, NCORES], F32, tag="thrM")
        nc.vector.memset(thrM, 127.5)
        nc.vector.memset(thrM[:, 7:8], float(Q0 - 7 * WIN) + 1.5)
        vmask = work.tile([128, NCORES], F32, tag="vmask")
        nc.vector.tensor_tensor(out=vmask, in0=iotaL, in1=thrM, op=OP.is_le)
        lowM = work.tile([128, NCORES], F32, tag="lowM")
        nc.vector.tensor_scalar(out=lowM, in0=iotaL, scalar1=1.5,
                                scalar2=None, op0=OP.is_gt)
        nc.vector.tensor_mul(vmask, vmask, lowM)
        padM = work.tile([128, NCORES], U32, tag="padM")
        nc.vector.tensor_scalar(out=padM, in0=vmask, scalar1=-1.0,
                                scalar2=1.0, op0=OP.mult, op1=OP.add)
        pm3 = work.tile([128, 1], U32, tag="pm3")
        nc.vector.tensor_scalar(out=pm3, in0=iotaL[:, 0:1],
                                scalar1=float(W3) - 0.5,
                                scalar2=None, op0=OP.is_gt)

        # G1-derived diagonal prep (runs inside the collective-wait window):
        # cbt[i] = [c1 | b] for S block i, dS[i] = D1 diag, dgblk[i] = dS*I,
        # plus the scaled diag tiles that let NS(0)/cheb(0) start straight
        # from the PSUM accumulators.
        alpha = 2.0 / (LO + HI)
        cheb_sc = 2.0 / (HI - LO)
        cbt = [work.tile([128, 2], F32, tag=f"cb{i}", name=f"cb{i}")
               for i in range(NBLK)]
        dSl = [work.tile([128, 1], F32, tag=f"dS{i}", name=f"dS{i}")
               for i in range(NBLK)]
        dgblk = [work.tile([128, 128], BF16, tag=f"dg{i}", name=f"dg{i}")
                 for i in range(NBLK)]
        dgA0 = work.tile([128, 128], BF16, tag="dgA0")
        dgS0 = work.tile([128, 128], BF16, tag="dgS0")
        with tc.tile_pool(name="g1_ps", bufs=2,
                          space=bass.MemorySpace.PSUM) as gps1:
            for i in range(NBLK):
                wi = 128 if i < NBLK - 1 else W3
                psT = gps1.tile([128, 2], F32, tag="pst")
                nc.tensor.transpose(psT[:wi, :], G1[0:2, ds(i * 128, wi)],
                                    ident[0:2, 0:2])
                nc.vector.memset(cbt[i], 0.0)
                nc.vector.tensor_copy(cbt[i][:wi, :], psT[:wi, :])
                nc.vector.tensor_scalar(out=dSl[i], in0=cbt[i][:, 0:1],
                                        scalar1=cst[:, 3:4],
                                        scalar2=None, op0=OP.add)
                if i == NBLK - 1:
                    nc.vector.copy_predicated(dSl[i], pm3, ones512[:, 0:1])
                nc.vector.tensor_scalar_mul(dgblk[i], ident, dSl[i])
        nc.vector.tensor_scalar_mul(dgA0, dgblk[0], alpha)
        nc.vector.tensor_scalar_mul(dgS0, dgblk[0], cheb_sc)
        nc.vector.tensor_sub(dgS0, dgS0, shiftI)
        cI = []
        for j in range(NCOEF):
            cj = work.tile([128, 128], BF16, tag=f"cI{j}", name=f"cI{j}")
            nc.vector.tensor_scalar_mul(cj, ident, chbB[:, j:j + 1])
            cI.append(cj)

        # ---- S assembly from the gathered buffer, tile-pipelined ----
        Avec = work.tile([128, NCORES], F32, tag="Avec")
        aAll = work.tile([128, NCORES], F32, tag="aAll")
        Winv = work.tile([128, NCORES], F32, tag="Winv")
        aW = work.tile([128, NCORES], F32, tag="aW")
        Srow = [work.tile([128, SP], BF16, tag=f"Srow{i}", name=f"Srow{i}")
                for i in range(NBLK)]
        zvec = [work.tile([128, 1], F32, tag=f"z{i}", name=f"z{i}")
                for i in range(NBLK)]

        Binv = [work.tile([128, 128], F32, tag=f"Binv{k}", name=f"Binv{k}")
                for k in range(NBLK)]
        Wk = [work.tile([128, SP - (k + 1) * 128], BF16, tag=f"Wk{k}",
                        name=f"Wk{k}") for k in range(NBLK - 1)]
        Wk32 = [work.tile([128, SP - (k + 1) * 128], F32, tag=f"Wk32_{k}",
                          name=f"Wk32_{k}") for k in range(NBLK - 1)]
        trcred = work.tile([128, NBLK * NCOEF], F32, tag="trcred")
        trc = work.tile([128, NBLK], F32, tag="trc")
        qtt = work.tile([128, NBLK], F32, tag="qtt")

        with (
            tc.tile_pool(name="sasm", bufs=2) as sasm,
            tc.tile_pool(name="sasm_ps", bufs=1,
                         space=bass.MemorySpace.PSUM) as sps,
        ):
            psS = [sps.tile([128, Q1 + 1], F32, tag=f"psS{i}", name=f"psS{i}")
                   for i in range(NBLK)]
            for t in range(NCORES):
                tt = t % 2
                gt8 = sasm.tile([128, AGW], I8, tag=f"gt8_{tt}",
                                name=f"gt8_{tt}")
                nc.sync.dma_start(
                    gt8, agb_out[t * 128 * AGW:(t + 1) * 128 * AGW]
                    .rearrange("(p f) -> p f", p=128))
                Ct = sasm.tile([128, Q1], BF16, tag=f"Ct_{tt}",
                               name=f"Ct_{tt}")
                if t % 2:
                    nc.scalar.copy(Ct, gt8[:, 0:Q1])
                else:
                    nc.vector.tensor_copy(Ct, gt8[:, 0:Q1])
                nc.vector.tensor_copy(Avec[:, t:t + 1], gt8[:, Q1:Q1 + 1])
                nc.vector.tensor_copy(aAll[:, t:t + 1],
                                      gt8[:, 504:508].bitcast(F32))
                nc.vector.tensor_scalar(out=Avec[:, t:t + 1],
                                        in0=Avec[:, t:t + 1],
                                        scalar1=cst[:, 2:3],
                                        scalar2=None, op0=OP.add)
                nc.vector.copy_predicated(Avec[:, t:t + 1], padM[:, t:t + 1],
                                          ones512[:, 0:1])
                nc.vector.reciprocal(Winv[:, t:t + 1], Avec[:, t:t + 1])
                nc.vector.tensor_mul(Winv[:, t:t + 1], Winv[:, t:t + 1],
                                     vmask[:, t:t + 1])
                nc.vector.tensor_mul(aW[:, t:t + 1], aAll[:, t:t + 1],
                                     Winv[:, t:t + 1])
                rhw = sasm.tile([128, Q1 + 1], BF16, tag=f"rhw_{tt}",
                                name=f"rhw_{tt}")
                nc.vector.tensor_scalar_mul(rhw[:, 0:Q1], Ct, Winv[:, t:t + 1])
                nc.vector.tensor_copy(rhw[:, Q1:Q1 + 1], aW[:, t:t + 1])
                for i in range(NBLK):
                    wi = 128 if i < NBLK - 1 else W3
                    nc.tensor.matmul(psS[i][:wi, :], Ct[:, ds(i * 128, wi)],
                                     rhw, start=(t == 0),
                                     stop=(t == NCORES - 1))

            # logA and quad_a (off the matmul critical path)
            scr_t = work.tile([128, NCORES], F32, tag="scr_t")
            logA = work.tile([128, 1], F32, tag="logA")
            nc.scalar.activation(scr_t, Avec, ACT.Ln, accum_out=logA)
            qa = work.tile([128, 1], F32, tag="qa")
            nc.vector.tensor_mul(scr_t, aAll, aAll)
            nc.vector.tensor_mul(scr_t, scr_t, Winv)
            nc.vector.tensor_reduce(qa, scr_t, AX.X, OP.add)

            # Srow / zvec assembly from psS (diag prep tiles precomputed)
            for i in range(NBLK):
                wi = 128 if i < NBLK - 1 else W3
                if i == NBLK - 1:
                    nc.vector.memset(Srow[i], 0.0)
                else:
                    nc.vector.memset(Srow[i][:, Q1:SP], 0.0)
                nc.vector.tensor_scalar_mul(Srow[i][:wi, 0:Q1],
                                            psS[i][:wi, 0:Q1], -1.0)
                nc.vector.tensor_add(Srow[i][:, ts(i, 128)],
                                     Srow[i][:, ts(i, 128)], dgblk[i])
                nc.vector.memset(zvec[i], 0.0)
                nc.vector.tensor_sub(zvec[i][:wi, :], cbt[i][:wi, 1:2],
                                     psS[i][:wi, Q1:Q1 + 1])

        with (
            tc.tile_pool(name="ldl", bufs=4) as ldl,
            tc.tile_pool(name="ldl_ps", bufs=4,
                         space=bass.MemorySpace.PSUM) as lps,
            tc.tile_pool(name="rps_pool", bufs=1,
                         space=bass.MemorySpace.PSUM) as rpsp,
        ):
            # ---- block LDL (Hotelling/NS inverses) with the Chebyshev
            # trace recurrences emission-interleaved into the NS bubbles.
            # Traces via tensor_tensor_reduce on DVE (no PE accumulation).
            cheb = {}          # k -> [b2, tprev, tcur, next_j]
            pending = []
            Rps = []

            def cheb_start(k):
                bh = ldl.tile([128, 128], BF16, tag=f"bh{k}", name=f"bh{k}")
                Bk = Srow[k][:, ts(k, 128)]
                nc.vector.tensor_scalar_mul(bh, Bk, cheb_sc)
                nc.vector.tensor_sub(bh, bh, shiftI)
                b2 = ldl.tile([128, 128], BF16, tag=f"b2{k}", name=f"b2{k}")
                nc.vector.tensor_scalar_mul(b2, bh, 2.0)
                R = rpsp.tile([128, 128], F32, tag=f"Rps{k}", name=f"Rps{k}")
                Rps.append(R)
                nc.tensor.matmul(R, cI[1], bh, start=True, stop=False)
                cheb[k] = [b2, identB16, bh, 2]
                pending.append(k)

            def cheb_round():
                if not pending:
                    return
                k = pending.pop(0)
                b2, tprev, tcur, j = cheb[k]
                psc = lps.tile([128, 128], F32, tag="lps")
                nc.tensor.matmul(psc, b2, tcur, start=True, stop=True)
                tnext = ldl.tile([128, 128], BF16, tag=f"chT{k}",
                                 name=f"chT{k}_{j}", bufs=3)
                nc.vector.tensor_sub(tnext, psc, tprev)
                nc.tensor.matmul(Rps[k], cI[j], tnext, start=False,
                                 stop=(j == CHEB_DEG))
                cheb[k] = [b2, tcur, tnext, j + 1]
                if j + 1 <= CHEB_DEG:
                    pending.append(k)

            for k in range(NBLK):
                # D_k is final here (panel k-1 updates already applied)
                cheb_start(k)
                trail = SP - (k + 1) * 128 if k < NBLK - 1 else 0
                Y = ldl.tile([128, 128], BF16, tag="nsY")
                nc.vector.tensor_scalar_mul(Y, Srow[k][:, ts(k, 128)], alpha)
                Z = ldl.tile([128, 128], BF16, tag="nsZ")
                nc.vector.tensor_sub(Z, i2, Y)
                X = ldl.tile([128, 128], BF16, tag="nsX")
                nc.vector.tensor_copy(X, alphaI)
                psX = None
                for it in range(NS_ITERS):
                    last = it == NS_ITERS - 1
                    if not last:
                        psY = lps.tile([128, 128], F32, tag="lps")
                        nc.tensor.matmul(psY, Y, Z, start=True, stop=True)
                    psX = lps.tile([128, 128], F32, tag="lps")
                    nc.tensor.matmul(psX, X, Z, start=True, stop=True)
                    cheb_round()
                    X = ldl.tile([128, 128], BF16, tag="nsX")
                    nc.vector.tensor_copy(X, psX)
                    if not last:
                        Z = ldl.tile([128, 128], BF16, tag="nsZ")
                        nc.vector.tensor_sub(Z, i2, psY)
                        Y = ldl.tile([128, 128], BF16, tag="nsY")
                        nc.vector.tensor_copy(Y, psY)
                nc.vector.tensor_copy(Binv[k], psX)
                if trail:
                    psW = lps.tile([128, 384], F32, tag="lps")
                    nc.tensor.matmul(psW[:, :trail], X,
                                     Srow[k][:, (k + 1) * 128:SP],
                                     start=True, stop=True)
                    nc.vector.tensor_copy(Wk[k], psW[:, :trail])
                    nc.vector.tensor_copy(Wk32[k], psW[:, :trail])
                    for i in range(k + 1, NBLK):
                        psu = lps.tile([128, 384], F32, tag="lps")
                        nc.tensor.matmul(psu[:, :trail], Srow[k][:, ts(i, 128)],
                                         Wk[k], start=True, stop=True)
                        nc.vector.tensor_sub(Srow[i][:, (k + 1) * 128:SP],
                                             Srow[i][:, (k + 1) * 128:SP],
                                             psu[:, :trail])
                    cheb_round()
                    cheb_round()
            while pending:
                cheb_round()
            for k in range(NBLK):
                Rsb = ldl.tile([128, 128], F32, tag="Rsb")
                nc.vector.tensor_mul(Rsb, Rps[k], ident)   # diagonal only
                nc.vector.tensor_reduce(trc[:, k:k + 1], Rsb, AX.X, OP.add)

            # forward substitution: z_i -= (Wk[k] block i)^T z_k
            for k in range(NBLK - 1):
                for i in range(k + 1, NBLK):
                    psz = lps.tile([128, 1], F32, tag="lps")
                    off = (i - k - 1) * 128
                    nc.tensor.matmul(psz, Wk32[k][:, ds(off, 128)], zvec[k],
                                     start=True, stop=True)
                    nc.vector.tensor_sub(zvec[i], zvec[i], psz)
            # quad_t = sum_k z_k^T Binv_k z_k
            for k in range(NBLK):
                psq = lps.tile([128, 1], F32, tag="lps")
                nc.tensor.matmul(psq, Binv[k], zvec[k], start=True, stop=True)
                uk = ldl.tile([128, 1], F32, tag="uk")
                nc.vector.tensor_copy(uk, psq)
                nc.vector.tensor_mul(qtt[:, k:k + 1], zvec[k], uk)

        # ---- final scalar assembly ----
        qtr = work.tile([128, 1], F32, tag="qtr")
        nc.vector.tensor_reduce(qtr, qtt, AX.X, OP.add)
        smalls_c = work.tile([128, 3 + NBLK], F32, tag="smalls_c")
        nc.vector.tensor_copy(smalls_c[:, 0:1], logA)
        nc.vector.tensor_copy(smalls_c[:, 1:2], qa)
        nc.vector.tensor_copy(smalls_c[:, 2:3], qtr)
        nc.vector.tensor_copy(smalls_c[:, 3:3 + NBLK], trc)
        smalls = work.tile([1, 3 + NBLK], F32, tag="smalls")
        ldS = work.tile([1, 1], F32, tag="ldS")
        with tc.tile_pool(name="fin_ps", bufs=1,
                          space=bass.MemorySpace.PSUM) as gps2:
            ps_sm = gps2.tile([128, 3 + NBLK], F32, tag="gps2")
            nc.tensor.matmul(ps_sm[0:1, :], ones512[:, 0:1], smalls_c,
                             start=True, stop=True)
            nc.vector.tensor_copy(smalls, ps_sm[0:1, :])
        nc.vector.tensor_reduce(ldS, smalls[:, 3:3 + NBLK], AX.X, OP.add)

        fin = work.tile([1, 8], F32, tag="fin")
        mtm = scal2r[:, 1:2]
        r2g = scal2r[:, 0:1]
        # quadK = quad_a + quad_t
        nc.vector.tensor_add(fin[:, 0:1], smalls[:, 1:2], smalls[:, 2:3])
        # mVinvm = (sig2/sig2e) * (mtm - quadK)
        nc.vector.tensor_sub(fin[:, 1:2], mtm, fin[:, 0:1])
        nc.vector.tensor_scalar_mul(fin[:, 1:2], fin[:, 1:2], cst[0:1, 6:7])
        # logdetV = const1 + sum log A + logdet S
        nc.vector.tensor_add(fin[:, 2:3], smalls[:, 0:1], ldS)
        nc.vector.tensor_scalar(out=fin[:, 2:3], in0=fin[:, 2:3],
                                scalar1=cst[0:1, 4:5], scalar2=None, op0=OP.add)
        # sum_log_pdf = const2 - sum_r2/(2 sig2)
        nc.vector.tensor_scalar(out=fin[:, 3:4], in0=r2g, scalar1=cst[0:1, 7:8],
                                scalar2=cst[0:1, 5:6], op0=OP.mult, op1=OP.add)
        # total = 0.5*(logdetV + mVinvm - mtm + sum_log_pdf)
        nc.vector.tensor_add(fin[:, 4:5], fin[:, 2:3], fin[:, 1:2])
        nc.vector.tensor_sub(fin[:, 4:5], fin[:, 4:5], mtm)
        nc.vector.tensor_add(fin[:, 4:5], fin[:, 4:5], fin[:, 3:4])
        nc.vector.tensor_scalar_mul(fin[:, 4:5], fin[:, 4:5], 0.5)

        nc.sync.dma_start(out_d[:], fin[:, 4:5])

    nc.finalize()
    return nc


def host_consts(sig2e, sig2bs, core_id):
    s0, s1 = float(sig2bs[0]), float(sig2bs[1])
    sig2e = float(sig2e)
    sig2 = sig2e + s0 + s1
    c = np.zeros(16, np.float32)
    c[0] = 1.0 / math.sqrt(sig2)
    c[1] = CLIP
    c[2] = sig2e / s0
    c[3] = sig2e / s1
    # the j=0 Chebyshev trace term (c_0 * 128 per block) folds in here
    c[4] = ((N - Q0 - Q1) * math.log(sig2e) + Q0 * math.log(s0)
            + Q1 * math.log(s1) - N * math.log(sig2)
            + NBLK * 128 * float(cheb_coeffs()[0]))
    c[5] = -0.5 * N * math.log(2.0 * math.pi * sig2)
    c[6] = sig2 / sig2e
    c[7] = -1.0 / (2.0 * sig2)
    c[8] = -CLIP
    c[9] = float(WIN * core_id)
    return c


_CACHE = {}


def _get_module(n_cores=NCORES):
    if n_cores not in _CACHE:
        _CACHE[n_cores] = build_module(n_cores)
    return _CACHE[n_cores]


def make_in_maps(inputs, n_cores=NCORES):
    y_true = np.ascontiguousarray(np.asarray(inputs["y_true"], np.float32).reshape(N, 1))
    y_pred = np.ascontiguousarray(np.asarray(inputs["y_pred"], np.float32).reshape(N, 1))
    zi0 = np.ascontiguousarray(np.asarray(inputs["Z_idx0"]).astype(np.int32).reshape(N))
    zi1 = np.ascontiguousarray(np.asarray(inputs["Z_idx1"]).astype(np.int32).reshape(N))
    cs = cheb_coeffs().astype(np.float32)
    chebc = np.concatenate([cs, np.zeros(NCOEF, np.float32)])
    pk = np.concatenate([
        y_true.reshape(NCH, 128).T,
        y_pred.reshape(NCH, 128).T,
        zi0.reshape(NCH, 128).T.view(np.float32),
        zi1.reshape(NCH, 128).T.view(np.float32),
    ], axis=1)
    pk = np.ascontiguousarray(pk)
    maps = []
    for i in range(n_cores):
        c = host_consts(np.asarray(inputs["sig2e"]),
                        np.asarray(inputs["sig2bs"], np.float64), i)
        maps.append({"packed": pk, "consts": c, "chebc": chebc})
    return maps


def kernel(**inputs):
    nc = _get_module(NCORES)
    maps = make_in_maps(inputs, NCORES)
    res = run_bass_kernel_spmd(nc, maps, list(range(NCORES)))
    out = np.asarray(res.results[0]["out"], np.float32).reshape(1, 1)
    return out


# revision 26
# speedup vs baseline: 1.2981x; 1.2981x over previous
"""COPNLL loss kernel for Trainium2 (8 NeuronCores) — v2 (slice scheme).

Math: the reference builds V = (sig2e*I + sig2bs0*Z0 Z0^T + sig2bs1*Z1 Z1^T)/sig2
with Z0 (4096x1000), Z1 (4096x500) one-hot, then needs logdet(V) and m^T V^-1 m.
Both reduce via Woodbury to the 1500x1500 capacitance matrix whose (0,0) block
is diagonal, leaving one dense 500x500 Schur complement
    S = (sig2e/s1*I + diag(c1)) - C^T diag(1/A) C,   A = sig2e/s0 + c0
with C = Z0^T Z1 (co-occurrence counts), c0/c1 level counts, a = Z0^T m, b = Z1^T m:
    logdet(sig2*V) = (N-q)log sig2e + q0 log s0 + q1 log s1 + sum(log A) + logdet S
    m^T V^-1 m     = (sig2/sig2e) * (m^T m - a^T A^-1 a - t^T S^-1 t),
                     t = b - C^T (a/A)

Device plan (SPMD, 8 cores, FULL inputs replicated to every core):
  phase A: core p owns the 126-level window [126p, 126p+126) of the level-0
    axis. Per 128-row chunk (32 chunks = all N rows) ONE matmul with the
    stationary matrix [onehot0_window | 1 | m] (128 cols) against
    [onehot1 | 1 | m] (502 cols) accumulates, over all rows:
      rows 0..125: C slice | counts0 slice | a slice
      rows 126/127: the full G1 = [1|m]^T[Z1|1|m]  (identical on all cores)
    so counts0/a/G1/C all come from one PSUM bank. m, m^Tm, r^Tr are local.
  comm: ONE int8 AllGather (64KB/core): cols 0..500 = int8 C+counts slice,
    cols 504..507 = f32 a slice bit-cast (AllGather concatenates, bit-exact).
    A garbage warmup AllGather issued at t=0 pre-pays the NRT rendezvous
    barrier concurrently with phase A.
  phase C (redundant on all cores): S assembled tile-by-tile from the gathered
    buffer (the t-vector rides as rhs column 500); block LDL with 128-blocks;
    block inverses via Hotelling/Newton-Schulz; block logdets via Chebyshev
    trace of log accumulated on the PE.
"""

import math
import sys
import types

import numpy as np

import concourse.bass as bass
import concourse.bacc as bacc
import concourse.mybir as mybir
from concourse.bass import ds, ts
from concourse.bass_utils import run_bass_kernel_spmd
from concourse.masks import make_identity
from concourse.tile import TileContext


def _ensure_axon_hooks():
    """bass_utils imports antenv.axon_hooks when tracing; this image's antenv
    lacks it. Provide a shim (with the real ctypes NTFF hook when available)
    so trace=True/BASS_TRACE never crashes the kernel."""
    try:
        import antenv.axon_hooks  # noqa: F401
        return
    except ImportError:
        pass
    try:
        import trn_agent_boot.trn_boot as tb
        hook = tb._ntff_profile_via_ctypes("/opt/axon/libaxon_pjrt.so")
    except Exception:
        hook = None
    mod = types.ModuleType("antenv.axon_hooks")
    mod._hook = hook
    mod.get_axon_ntff_profile_hook = lambda: mod._hook

    def _set(h):
        mod._hook = h

    mod.set_axon_ntff_profile_hook = _set
    sys.modules["antenv.axon_hooks"] = mod
    try:
        import antenv
        antenv.axon_hooks = mod
    except ImportError:
        pass
    try:
        import concourse.bass_utils as bu
        _orig_upload = bu.upload_artifacts

        def _safe_upload(tmpdir):
            try:
                return _orig_upload(tmpdir)
            except Exception:
                return f"local:{tmpdir}"

        bu.upload_artifacts = _safe_upload
    except Exception:
        pass


_ensure_axon_hooks()

N = 4096
NCORES = 8
NCH = N // 128             # 32 row chunks, every core sees all of them
WIN = 126                  # level-0 window width per core (8*126=1008 >= 1000)
Q0 = 1000
Q1 = 500
FR = Q1 + 2                # rhs width: [Z1 | 1 | m]
AGW = 512                  # allgather row width (int8)
SP = 512                   # padded S size
NBLK = SP // 128           # 4
W3 = Q1 - 3 * 128          # 116: valid width of the last S block
LO, HI = 1.4, 18.0         # eigenvalue bounds for NS init + Chebyshev interval
NS_ITERS = 3
CHEB_DEG = 6
NCOEF = CHEB_DEG + 1
CLIP = 4.2648907939226017  # sqrt(2)*erfinv(1-2e-5)

F32 = mybir.dt.float32
BF16 = mybir.dt.bfloat16
I8 = mybir.dt.int8
I32 = mybir.dt.int32
U32 = mybir.dt.uint32
AX = mybir.AxisListType
OP = mybir.AluOpType
ACT = mybir.ActivationFunctionType


def cheb_coeffs(lo=LO, hi=HI, deg=CHEB_DEG):
    K = 4000
    th = (np.arange(K) + 0.5) * np.pi / K
    xk = np.cos(th)
    fk = np.log((hi - lo) / 2.0 * xk + (hi + lo) / 2.0)
    cs = np.array([2.0 / K * np.sum(fk * np.cos(j * th)) for j in range(deg + 1)])
    cs[0] *= 0.5
    return cs.astype(np.float32)


def _diag_fill(nc, tile_ap, value):
    nc.gpsimd.memset(tile_ap, 0.0)
    nc.gpsimd.affine_select(out=tile_ap, in_=tile_ap, compare_op=OP.not_equal,
                            fill=value, base=0, pattern=[[-1, 128]],
                            channel_multiplier=1)


def build_module(n_cores=NCORES):
    nc = bacc.Bacc(num_devices=n_cores)
    pk_d = nc.declare_dram_parameter("packed", [128, 4 * NCH], F32,
                                     isOutput=False)
    cst_d = nc.declare_dram_parameter("consts", [16], F32, isOutput=False)
    chb_d = nc.declare_dram_parameter("chebc", [2 * NCOEF], F32, isOutput=False)
    out_d = nc.declare_dram_parameter("out", [1, 1], F32, isOutput=True)

    agb_in = nc.dram_tensor("agb_in", [128 * AGW], I8)
    agb_out = nc.dram_tensor("agb_out", [n_cores * 128 * AGW], I8,
                             addr_space="Shared")
    warm_in = nc.dram_tensor("warm_in", [16], F32)
    warm_out = nc.dram_tensor("warm_out", [16 * n_cores], F32,
                              addr_space="Shared")

    with TileContext(nc) as tc, \
         tc.tile_pool(name="consts", bufs=1) as consts, \
         tc.tile_pool(name="work", bufs=1) as work:

        # warmup collective: pre-pays the one-time cross-core rendezvous
        # barrier concurrently with phase A. Data is garbage; result unused.
        if n_cores > 1:
            nc.gpsimd.collective_compute(
                "AllGather", OP.bypass,
                replica_groups=[list(range(n_cores))],
                ins=[warm_in[:]], outs=[warm_out[:]],
            )

        # ---- constants ----
        ident = consts.tile([128, 128], F32, tag="ident")
        make_identity(nc, ident)
        i2 = consts.tile([128, 128], F32, tag="i2")              # 2*I
        _diag_fill(nc, i2, 2.0)
        alphaI = consts.tile([128, 128], F32, tag="alphaI")      # NS init
        _diag_fill(nc, alphaI, 2.0 / (LO + HI))
        shiftI = consts.tile([128, 128], F32, tag="shiftI")      # Chebyshev shift
        _diag_fill(nc, shiftI, (HI + LO) / (HI - LO))
        ones512 = consts.tile([128, SP], F32, tag="ones512")
        nc.vector.memset(ones512, 1.0)
        identB16 = consts.tile([128, 128], BF16, tag="identB16")
        nc.vector.tensor_copy(identB16, ident)

        cst_row = consts.tile([1, 16], F32, tag="cst_row")
        nc.sync.dma_start(cst_row, cst_d[:].rearrange("(p x) -> p x", p=1))
        cst_row2 = consts.tile([1, 16], F32, tag="cst_row2")
        nc.vector.tensor_copy(cst_row2, cst_row)
        chb = consts.tile([1, 2 * NCOEF], F32, tag="chb")
        nc.sync.dma_start(chb, chb_d[:].rearrange("(p x) -> p x", p=1))
        chb2 = consts.tile([1, 2 * NCOEF], F32, tag="chb2")
        nc.vector.tensor_copy(chb2, chb)
        cst = consts.tile([128, 16], F32, tag="cst")
        chbB = consts.tile([128, 2 * NCOEF], F32, tag="chbB")
        with tc.tile_pool(name="setup_ps", bufs=2,
                          space=bass.MemorySpace.PSUM) as gps0:
            ps_b = gps0.tile([128, 16], F32, tag="gps0")
            nc.tensor.matmul(ps_b, ones512[0:1, 0:128], cst_row2,
                             start=True, stop=True)
            nc.vector.tensor_copy(cst, ps_b)
            ps_c = gps0.tile([128, 2 * NCOEF], F32, tag="gps0")
            nc.tensor.matmul(ps_c, ones512[0:1, 0:128], chb2,
                             start=True, stop=True)
            nc.vector.tensor_copy(chbB, ps_c)

        # iotas for the one-hot compares
        iota1i = work.tile([128, Q1], I32, tag="iota1i")
        nc.gpsimd.iota(iota1i, pattern=[[1, Q1]], base=0, channel_multiplier=0)
        iota1 = work.tile([128, Q1], F32, tag="iota1")
        nc.vector.tensor_copy(iota1, iota1i)
        iotaWi = work.tile([128, WIN], I32, tag="iotaWi")
        nc.gpsimd.iota(iotaWi, pattern=[[1, WIN]], base=0, channel_multiplier=0)
        iotaW = work.tile([128, WIN], F32, tag="iotaW")
        nc.vector.tensor_copy(iotaW, iotaWi)
        # shift by this core's window base (per-core const c[9])
        nc.vector.tensor_scalar(out=iotaW, in0=iotaW, scalar1=cst[:, 9:10],
                                scalar2=None, op0=OP.add)

        # ---- inputs -> m, sum r^2, sum m^2 ----
        packed = work.tile([128, 4 * NCH], F32, tag="packed")
        nc.sync.dma_start(packed, pk_d[:])
        yt = packed[:, 0:NCH]
        yp = packed[:, NCH:2 * NCH]
        idx0 = work.tile([128, NCH], F32, tag="idx0")
        nc.vector.tensor_copy(idx0, packed[:, 2 * NCH:3 * NCH].bitcast(I32))
        idx1 = work.tile([128, NCH], F32, tag="idx1")
        nc.vector.tensor_copy(idx1, packed[:, 3 * NCH:4 * NCH].bitcast(I32))
        resid = work.tile([128, NCH], F32, tag="resid")
        nc.vector.tensor_sub(resid, yt, yp)
        mvec = work.tile([128, NCH], F32, tag="mvec")
        nc.vector.tensor_scalar(out=mvec, in0=resid, scalar1=cst[:, 0:1],
                                scalar2=cst[:, 1:2], op0=OP.mult, op1=OP.min)
        nc.vector.tensor_scalar(out=mvec, in0=mvec, scalar1=cst[:, 8:9],
                                scalar2=None, op0=OP.max)
        scr_n = work.tile([128, NCH], F32, tag="scr_n")
        scal2 = work.tile([128, 2], F32, tag="scal2")
        nc.vector.tensor_mul(scr_n, resid, resid)
        nc.vector.tensor_reduce(scal2[:, 0:1], scr_n, AX.X, OP.add)
        nc.vector.tensor_mul(scr_n, mvec, mvec)
        nc.vector.tensor_reduce(scal2[:, 1:2], scr_n, AX.X, OP.add)
        scal2r = work.tile([1, 2], F32, tag="scal2r")
        with tc.tile_pool(name="sc_ps", bufs=1,
                          space=bass.MemorySpace.PSUM) as gpsc:
            ps_s = gpsc.tile([128, 2], F32, tag="gpsc")
            nc.tensor.matmul(ps_s[0:1, 0:2], ones512[:, 0:1], scal2,
                             start=True, stop=True)
            nc.vector.tensor_copy(scal2r, ps_s[0:1, 0:2])

        # ---- phase A: one matmul per 128-row chunk over all N rows ----
        G1 = work.tile([2, FR], F32, tag="G1")
        stg = work.tile([128, AGW], I8, tag="stg")
        with (
            tc.tile_pool(name="phA", bufs=3) as pha,
            tc.tile_pool(name="phA_ps", bufs=1, space=bass.MemorySpace.PSUM) as pps,
        ):
            psC = pps.tile([128, FR], F32, tag="psC")
            for c in range(NCH):
                cc = c % 3
                # cols 0/1 = [1|m] (G1 lands at psC rows 0/1: PSUM partition
                # reads must start at 0); cols 2..127 = level-window one-hot
                oh = pha.tile([128, 128], BF16, tag=f"oh_{cc}", name=f"oh_{cc}")
                nc.vector.memset(oh[:, 0:1], 1.0)
                nc.scalar.copy(oh[:, 1:2], mvec[:, c:c + 1])
                nc.vector.tensor_scalar(out=oh[:, 2:128], in0=iotaW,
                                        scalar1=idx0[:, c:c + 1],
                                        scalar2=None, op0=OP.is_equal)
                rh = pha.tile([128, FR], BF16, tag=f"rh_{cc}", name=f"rh_{cc}")
                nc.vector.tensor_scalar(out=rh[:, 0:Q1], in0=iota1,
                                        scalar1=idx1[:, c:c + 1],
                                        scalar2=None, op0=OP.is_equal)
                nc.vector.memset(rh[:, Q1:Q1 + 1], 1.0)
                nc.scalar.copy(rh[:, Q1 + 1:FR], mvec[:, c:c + 1])
                nc.tensor.matmul(psC, oh, rh, start=(c == 0),
                                 stop=(c == NCH - 1))
            # readout: G1 (rows 0/1), int8 staging for the AllGather
            nc.vector.tensor_copy(G1, psC[0:2, :])
            nc.vector.memset(stg[:, Q1 + 1:AGW], 0)
            nc.vector.tensor_copy(stg[:, 0:Q1 + 1], psC[:, 0:Q1 + 1])
            af32 = work.tile([128, 1], F32, tag="af32")
            nc.vector.tensor_copy(af32, psC[:, Q1 + 1:FR])
            nc.vector.tensor_copy(stg[:, 504:508], af32[:, 0:1].bitcast(I8))

        nc.sync.dma_start(
            agb_in[:].rearrange("(p f) -> p f", p=128), stg)
        if n_cores > 1:
            nc.gpsimd.collective_compute(
                "AllGather", OP.bypass,
                replica_groups=[list(range(n_cores))],
                ins=[agb_in[:]], outs=[agb_out[:]],
            )
        else:
            nc.sync.dma_start(agb_out[:], agb_in[:])

        # ---- phase C constants built while the collective is in flight ----
        iotaLi = work.tile([128, NCORES], I32, tag="iotaLi")
        nc.gpsimd.iota(iotaLi, pattern=[[0, NCORES]], base=0,
                       channel_multiplier=1)
        iotaL = work.tile([128, NCORES], F32, tag="iotaL")
        nc.vector.tensor_copy(iotaL, iotaLi)
        # per-tile valid rows: 2 <= row, row <= thr (level = 126t + row - 2;
        # rows 0/1 are G1 junk; tile 7 rows >= 120 are levels >= 1000 pad)
        thrM = work.tile([128, NCORES], F32, tag="thrM")
        nc.vector.memset(thrM, 127.5)
        nc.vector.memset(thrM[:, 7:8], float(Q0 - 7 * WIN) + 1.5)
        vmask = work.tile([128, NCORES], F32, tag="vmask")
        nc.vector.tensor_tensor(out=vmask, in0=iotaL, in1=thrM, op=OP.is_le)
        lowM = work.tile([128, NCORES], F32, tag="lowM")
        nc.vector.tensor_scalar(out=lowM, in0=iotaL, scalar1=1.5,
                                scalar2=None, op0=OP.is_gt)
        nc.vector.tensor_mul(vmask, vmask, lowM)
        padM = work.tile([128, NCORES], U32, tag="padM")
        nc.vector.tensor_scalar(out=padM, in0=vmask, scalar1=-1.0,
                                scalar2=1.0, op0=OP.mult, op1=OP.add)
        pm3 = work.tile([128, 1], U32, tag="pm3")
        nc.vector.tensor_scalar(out=pm3, in0=iotaL[:, 0:1],
                                scalar1=float(W3) - 0.5,
                                scalar2=None, op0=OP.is_gt)

        # G1-derived diagonal prep (runs inside the collective-wait window):
        # cbt[i] = [c1 | b] for S block i, dS[i] = D1 diag, dgblk[i] = dS*I,
        # plus the scaled diag tiles that let NS(0)/cheb(0) start straight
        # from the PSUM accumulators.
        alpha = 2.0 / (LO + HI)
        cheb_sc = 2.0 / (HI - LO)
        cbt = [work.tile([128, 2], F32, tag=f"cb{i}", name=f"cb{i}")
               for i in range(NBLK)]
        dSl = [work.tile([128, 1], F32, tag=f"dS{i}", name=f"dS{i}")
               for i in range(NBLK)]
        dgN = [work.tile([128, 128], BF16, tag=f"dg{i}", name=f"dg{i}")
               for i in range(NBLK)]
        with tc.tile_pool(name="g1_ps", bufs=2,
                          space=bass.MemorySpace.PSUM) as gps1:
            for i in range(NBLK):
                wi = 128 if i < NBLK - 1 else W3
                psT = gps1.tile([128, 2], F32, tag="pst")
                nc.tensor.transpose(psT[:wi, :], G1[0:2, ds(i * 128, wi)],
                                    ident[0:2, 0:2])
                nc.vector.memset(cbt[i], 0.0)
                nc.vector.tensor_copy(cbt[i][:wi, :], psT[:wi, :])
                nc.vector.tensor_scalar(out=dSl[i], in0=cbt[i][:, 0:1],
                                        scalar1=cst[:, 3:4],
                                        scalar2=None, op0=OP.add)
                if i == NBLK - 1:
                    nc.vector.copy_predicated(dSl[i], pm3, ones512[:, 0:1])
                    # block 3: pad-identity diag entries land at Srow cols
                    # >= 500, outside psS — keep the positive full-tile add
                    nc.vector.tensor_scalar_mul(dgN[i], ident, dSl[i])
                else:
                    # negated: folded into psS via a PE accumulation matmul
                    nc.vector.tensor_scalar(out=dSl[i], in0=dSl[i],
                                            scalar1=-1.0,
                                            scalar2=None, op0=OP.mult)
                    nc.vector.tensor_scalar_mul(dgN[i], ident, dSl[i])
        cI = []
        for j in range(NCOEF):
            cj = work.tile([128, 128], BF16, tag=f"cI{j}", name=f"cI{j}")
            nc.vector.tensor_scalar_mul(cj, ident, chbB[:, j:j + 1])
            cI.append(cj)

        # ---- S assembly from the gathered buffer, tile-pipelined ----
        Avec = work.tile([128, NCORES], F32, tag="Avec")
        aAll = work.tile([128, NCORES], F32, tag="aAll")
        Winv = work.tile([128, NCORES], F32, tag="Winv")
        Srow = [work.tile([128, SP], BF16, tag=f"Srow{i}", name=f"Srow{i}")
                for i in range(NBLK)]
        zvec = [work.tile([128, 1], F32, tag=f"z{i}", name=f"z{i}")
                for i in range(NBLK)]

        Binv = [work.tile([128, 128], F32, tag=f"Binv{k}", name=f"Binv{k}")
                for k in range(NBLK)]
        Wk = [work.tile([128, SP - (k + 1) * 128], BF16, tag=f"Wk{k}",
                        name=f"Wk{k}") for k in range(NBLK - 1)]
        Wk32 = [work.tile([128, SP - (k + 1) * 128], F32, tag=f"Wk32_{k}",
                          name=f"Wk32_{k}") for k in range(NBLK - 1)]
        trc = work.tile([128, NBLK], F32, tag="trc")
        qtt = work.tile([128, NBLK], F32, tag="qtt")

        with (
            tc.tile_pool(name="sasm", bufs=2) as sasm,
            tc.tile_pool(name="sasm_ps", bufs=1,
                         space=bass.MemorySpace.PSUM) as sps,
        ):
            # per-tile strided fetch of the counts / bitcast-a columns
            # (tiny DMAs on the sync queue: zero DVE ops)
            cnt8 = work.tile([128, NCORES], I8, tag="cnt8")
            a8 = work.tile([128, 4 * NCORES], I8, tag="a8")
            for t in range(NCORES):
                blk = agb_out[t * 128 * AGW:(t + 1) * 128 * AGW] \
                    .rearrange("(p f) -> p f", p=128)
                nc.sync.dma_start(cnt8[:, t:t + 1], blk[:, Q1:Q1 + 1])
                nc.sync.dma_start(a8[:, 4 * t:4 * t + 4], blk[:, 504:508])
            nc.vector.tensor_scalar(out=Avec, in0=cnt8, scalar1=cst[:, 2:3],
                                    scalar2=None, op0=OP.add)
            nc.vector.copy_predicated(Avec, padM, ones512[:, 0:NCORES])
            nc.vector.reciprocal(Winv, Avec)
            nc.vector.tensor_mul(Winv, Winv, vmask)
            ws = work.tile([128, NCORES], F32, tag="ws")
            nc.scalar.sqrt(ws, Winv)
            xw = work.tile([128, NCORES], F32, tag="xw")
            nc.vector.tensor_copy(aAll, a8[:, 0:4 * NCORES].bitcast(F32))
            nc.vector.tensor_mul(xw, aAll, ws)
            psS = [sps.tile([128, Q1 + 1], F32, tag=f"psS{i}", name=f"psS{i}")
                   for i in range(NBLK)]
            for t in range(NCORES):
                tt = t % 2
                gt8 = sasm.tile([128, AGW], I8, tag=f"gt8_{tt}",
                                name=f"gt8_{tt}")
                nc.sync.dma_start(
                    gt8, agb_out[t * 128 * AGW:(t + 1) * 128 * AGW]
                    .rearrange("(p f) -> p f", p=128))
                # Ch = sqrt(1/A)-scaled C slice (bf16) — used on BOTH sides
                rhw = sasm.tile([128, Q1 + 1], BF16, tag=f"rhw_{tt}",
                                name=f"rhw_{tt}")
                nc.vector.tensor_scalar_mul(rhw[:, 0:Q1], gt8[:, 0:Q1],
                                            ws[:, t:t + 1])
                nc.vector.tensor_copy(rhw[:, Q1:Q1 + 1], xw[:, t:t + 1])
                for i in range(NBLK):
                    wi = 128 if i < NBLK - 1 else W3
                    nc.tensor.matmul(psS[i][:wi, :], rhw[:, ds(i * 128, wi)],
                                     rhw, start=(t == 0),
                                     stop=(t == NCORES - 1))
                if t == 0:
                    # fold the -D1 diagonal into the accumulation group
                    for i in range(NBLK - 1):
                        nc.tensor.matmul(psS[i][:128, ds(i * 128, 128)],
                                         dgN[i], identB16,
                                         start=False, stop=False)

            # logA and quad_a (off the matmul critical path)
            scr_t = work.tile([128, NCORES], F32, tag="scr_t")
            logA = work.tile([128, 1], F32, tag="logA")
            nc.scalar.activation(scr_t, Avec, ACT.Ln, accum_out=logA)
            qa = work.tile([128, 1], F32, tag="qa")
            nc.vector.tensor_mul(scr_t, xw, xw)
            nc.vector.tensor_reduce(qa, scr_t, AX.X, OP.add)

            # Srow / zvec assembly from psS (diag prep tiles precomputed)
            for i in range(NBLK):
                wi = 128 if i < NBLK - 1 else W3
                if i == NBLK - 1:
                    nc.vector.memset(Srow[i], 0.0)
                else:
                    nc.vector.memset(Srow[i][:, Q1:SP], 0.0)
                nc.vector.tensor_scalar_mul(Srow[i][:wi, 0:Q1],
                                            psS[i][:wi, 0:Q1], -1.0)
                if i == NBLK - 1:
                    nc.vector.tensor_add(Srow[i][:, ts(i, 128)],
                                         Srow[i][:, ts(i, 128)], dgN[i])
                nc.vector.memset(zvec[i], 0.0)
                nc.vector.tensor_sub(zvec[i][:wi, :], cbt[i][:wi, 1:2],
                                     psS[i][:wi, Q1:Q1 + 1])

        with (
            tc.tile_pool(name="ldl", bufs=4) as ldl,
            tc.tile_pool(name="ldl_ps", bufs=4,
                         space=bass.MemorySpace.PSUM) as lps,
            tc.tile_pool(name="rps_pool", bufs=1,
                         space=bass.MemorySpace.PSUM) as rpsp,
        ):
            # ---- block LDL (Hotelling/NS inverses) with the Chebyshev
            # trace recurrences emission-interleaved into the NS bubbles.
            # Traces via tensor_tensor_reduce on DVE (no PE accumulation).
            cheb = {}          # k -> [b2, tprev, tcur, next_j]
            pending = []
            Rps = []

            def cheb_start(k):
                bh = ldl.tile([128, 128], BF16, tag=f"bh{k}", name=f"bh{k}")
                Bk = Srow[k][:, ts(k, 128)]
                nc.vector.tensor_scalar_mul(bh, Bk, cheb_sc)
                nc.vector.tensor_sub(bh, bh, shiftI)
                b2 = ldl.tile([128, 128], BF16, tag=f"b2{k}", name=f"b2{k}")
                nc.vector.tensor_scalar_mul(b2, bh, 2.0)
                R = rpsp.tile([128, 128], F32, tag=f"Rps{k}", name=f"Rps{k}")
                Rps.append(R)
                nc.tensor.matmul(R, cI[1], bh, start=True, stop=False)
                cheb[k] = [b2, identB16, bh, 2]
                pending.append(k)

            def cheb_round():
                if not pending:
                    return
                k = pending.pop(0)
                b2, tprev, tcur, j = cheb[k]
                psc = lps.tile([128, 128], F32, tag="lps")
                nc.tensor.matmul(psc, b2, tcur, start=True, stop=True)
                tnext = ldl.tile([128, 128], BF16, tag=f"chT{k}",
                                 name=f"chT{k}_{j}", bufs=3)
                nc.vector.tensor_sub(tnext, psc, tprev)
                nc.tensor.matmul(Rps[k], cI[j], tnext, start=False,
                                 stop=(j == CHEB_DEG))
                cheb[k] = [b2, tcur, tnext, j + 1]
                if j + 1 <= CHEB_DEG:
                    pending.append(k)

            for k in range(NBLK):
                # D_k is final here (panel k-1 updates already applied)
                cheb_start(k)
                trail = SP - (k + 1) * 128 if k < NBLK - 1 else 0
                Y = ldl.tile([128, 128], BF16, tag="nsY")
                nc.vector.tensor_scalar_mul(Y, Srow[k][:, ts(k, 128)], alpha)
                Z = ldl.tile([128, 128], BF16, tag="nsZ")
                nc.vector.tensor_sub(Z, i2, Y)
                X = ldl.tile([128, 128], BF16, tag="nsX")
                nc.vector.tensor_copy(X, alphaI)
                psX = None
                for it in range(NS_ITERS):
                    last = it == NS_ITERS - 1
                    if not last:
                        psY = lps.tile([128, 128], F32, tag="lps")
                        nc.tensor.matmul(psY, Y, Z, start=True, stop=True)
                    psX = lps.tile([128, 128], F32, tag="lps")
                    nc.tensor.matmul(psX, X, Z, start=True, stop=True)
                    cheb_round()
                    X = ldl.tile([128, 128], BF16, tag="nsX")
                    nc.vector.tensor_copy(X, psX)
                    if not last:
                        Z = ldl.tile([128, 128], BF16, tag="nsZ")
                        nc.vector.tensor_sub(Z, i2, psY)
                        Y = ldl.tile([128, 128], BF16, tag="nsY")
                        nc.scalar.copy(Y, psY)
                nc.scalar.copy(Binv[k], psX)
                if trail:
                    psW = lps.tile([128, 384], F32, tag="lps")
                    nc.tensor.matmul(psW[:, :trail], X,
                                     Srow[k][:, (k + 1) * 128:SP],
                                     start=True, stop=True)
                    nc.vector.tensor_copy(Wk[k], psW[:, :trail])
                    nc.scalar.copy(Wk32[k], psW[:, :trail])
                    for i in range(k + 1, NBLK):
                        psu = lps.tile([128, 384], F32, tag="lps")
                        nc.tensor.matmul(psu[:, :trail], Srow[k][:, ts(i, 128)],
                                         Wk[k], start=True, stop=True)
                        nc.vector.tensor_sub(Srow[i][:, (k + 1) * 128:SP],
                                             Srow[i][:, (k + 1) * 128:SP],
                                             psu[:, :trail])
                    cheb_round()
                    cheb_round()
            while pending:
                cheb_round()
            for k in range(NBLK):
                Rsb = ldl.tile([128, 128], F32, tag="Rsb")
                nc.vector.tensor_mul(Rsb, Rps[k], ident)   # diagonal only
                nc.vector.tensor_reduce(trc[:, k:k + 1], Rsb, AX.X, OP.add)

            # forward substitution: z_i -= (Wk[k] block i)^T z_k
            for k in range(NBLK - 1):
                for i in range(k + 1, NBLK):
                    psz = lps.tile([128, 1], F32, tag="lps")
                    off = (i - k - 1) * 128
                    nc.tensor.matmul(psz, Wk32[k][:, ds(off, 128)], zvec[k],
                                     start=True, stop=True)
                    nc.vector.tensor_sub(zvec[i], zvec[i], psz)
            # quad_t = sum_k z_k^T Binv_k z_k
            for k in range(NBLK):
                psq = lps.tile([128, 1], F32, tag="lps")
                nc.tensor.matmul(psq, Binv[k], zvec[k], start=True, stop=True)
                uk = ldl.tile([128, 1], F32, tag="uk")
                nc.vector.tensor_copy(uk, psq)
                nc.vector.tensor_mul(qtt[:, k:k + 1], zvec[k], uk)

        # ---- final scalar assembly ----
        qtr = work.tile([128, 1], F32, tag="qtr")
        nc.vector.tensor_reduce(qtr, qtt, AX.X, OP.add)
        smalls_c = work.tile([128, 3 + NBLK], F32, tag="smalls_c")
        nc.vector.tensor_copy(smalls_c[:, 0:1], logA)
        nc.vector.tensor_copy(smalls_c[:, 1:2], qa)
        nc.vector.tensor_copy(smalls_c[:, 2:3], qtr)
        nc.vector.tensor_copy(smalls_c[:, 3:3 + NBLK], trc)
        smalls = work.tile([1, 3 + NBLK], F32, tag="smalls")
        ldS = work.tile([1, 1], F32, tag="ldS")
        with tc.tile_pool(name="fin_ps", bufs=1,
                          space=bass.MemorySpace.PSUM) as gps2:
            ps_sm = gps2.tile([128, 3 + NBLK], F32, tag="gps2")
            nc.tensor.matmul(ps_sm[0:1, :], ones512[:, 0:1], smalls_c,
                             start=True, stop=True)
            nc.vector.tensor_copy(smalls, ps_sm[0:1, :])
        nc.vector.tensor_reduce(ldS, smalls[:, 3:3 + NBLK], AX.X, OP.add)

        fin = work.tile([1, 8], F32, tag="fin")
        mtm = scal2r[:, 1:2]
        r2g = scal2r[:, 0:1]
        # quadK = quad_a + quad_t
        nc.vector.tensor_add(fin[:, 0:1], smalls[:, 1:2], smalls[:, 2:3])
        # mVinvm = (sig2/sig2e) * (mtm - quadK)
        nc.vector.tensor_sub(fin[:, 1:2], mtm, fin[:, 0:1])
        nc.vector.tensor_scalar_mul(fin[:, 1:2], fin[:, 1:2], cst[0:1, 6:7])
        # logdetV = const1 + sum log A + logdet S
        nc.vector.tensor_add(fin[:, 2:3], smalls[:, 0:1], ldS)
        nc.vector.tensor_scalar(out=fin[:, 2:3], in0=fin[:, 2:3],
                                scalar1=cst[0:1, 4:5], scalar2=None, op0=OP.add)
        # sum_log_pdf = const2 - sum_r2/(2 sig2)
        nc.vector.tensor_scalar(out=fin[:, 3:4], in0=r2g, scalar1=cst[0:1, 7:8],
                                scalar2=cst[0:1, 5:6], op0=OP.mult, op1=OP.add)
        # total = 0.5*(logdetV + mVinvm - mtm + sum_log_pdf)
        nc.vector.tensor_add(fin[:, 4:5], fin[:, 2:3], fin[:, 1:2])
        nc.vector.tensor_sub(fin[:, 4:5], fin[:, 4:5], mtm)
        nc.vector.tensor_add(fin[:, 4:5], fin[:, 4:5], fin[:, 3:4])
        nc.vector.tensor_scalar_mul(fin[:, 4:5], fin[:, 4:5], 0.5)

        nc.sync.dma_start(out_d[:], fin[:, 4:5])

    nc.finalize()
    return nc


def host_consts(sig2e, sig2bs, core_id):
    s0, s1 = float(sig2bs[0]), float(sig2bs[1])
    sig2e = float(sig2e)
    sig2 = sig2e + s0 + s1
    c = np.zeros(16, np.float32)
    c[0] = 1.0 / math.sqrt(sig2)
    c[1] = CLIP
    c[2] = sig2e / s0
    c[3] = sig2e / s1
    # the j=0 Chebyshev trace term (c_0 * 128 per block) folds in here
    c[4] = ((N - Q0 - Q1) * math.log(sig2e) + Q0 * math.log(s0)
            + Q1 * math.log(s1) - N * math.log(sig2)
            + NBLK * 128 * float(cheb_coeffs()[0]))
    c[5] = -0.5 * N * math.log(2.0 * math.pi * sig2)
    c[6] = sig2 / sig2e
    c[7] = -1.0 / (2.0 * sig2)
    c[8] = -CLIP
    c[9] = float(WIN * core_id)
    return c


_CACHE = {}


def _get_module(n_cores=NCORES):
    if n_cores not in _CACHE:
        _CACHE[n_cores] = build_module(n_cores)
    return _CACHE[n_cores]


def make_in_maps(inputs, n_cores=NCORES):
    y_true = np.ascontiguousarray(np.asarray(inputs["y_true"], np.float32).reshape(N, 1))
    y_pred = np.ascontiguousarray(np.asarray(inputs["y_pred"], np.float32).reshape(N, 1))
    zi0 = np.ascontiguousarray(np.asarray(inputs["Z_idx0"]).astype(np.int32).reshape(N))
    zi1 = np.ascontiguousarray(np.asarray(inputs["Z_idx1"]).astype(np.int32).reshape(N))
    cs = cheb_coeffs().astype(np.float32)
    chebc = np.concatenate([cs, np.zeros(NCOEF, np.float32)])
    pk = np.concatenate([
        y_true.reshape(NCH, 128).T,
        y_pred.reshape(NCH, 128).T,
        zi0.reshape(NCH, 128).T.view(np.float32),
        zi1.reshape(NCH, 128).T.view(np.float32),
    ], axis=1)
    pk = np.ascontiguousarray(pk)
    maps = []
    for i in range(n_cores):
        c = host_consts(np.asarray(inputs["sig2e"]),
                        np.asarray(inputs["sig2bs"], np.float64), i)
        maps.append({"packed": pk, "consts": c, "chebc": chebc})
    return maps


def kernel(**inputs):
    nc = _get_module(NCORES)
    maps = make_in_maps(inputs, NCORES)
    res = run_bass_kernel_spmd(nc, maps, list(range(NCORES)))
    out = np.asarray(res.results[0]["out"], np.float32).reshape(1, 1)
    return out


# revision 27
# speedup vs baseline: 1.3353x; 1.0287x over previous
"""COPNLL loss kernel for Trainium2 (8 NeuronCores) — v2 (slice scheme).

Math: the reference builds V = (sig2e*I + sig2bs0*Z0 Z0^T + sig2bs1*Z1 Z1^T)/sig2
with Z0 (4096x1000), Z1 (4096x500) one-hot, then needs logdet(V) and m^T V^-1 m.
Both reduce via Woodbury to the 1500x1500 capacitance matrix whose (0,0) block
is diagonal, leaving one dense 500x500 Schur complement
    S = (sig2e/s1*I + diag(c1)) - C^T diag(1/A) C,   A = sig2e/s0 + c0
with C = Z0^T Z1 (co-occurrence counts), c0/c1 level counts, a = Z0^T m, b = Z1^T m:
    logdet(sig2*V) = (N-q)log sig2e + q0 log s0 + q1 log s1 + sum(log A) + logdet S
    m^T V^-1 m     = (sig2/sig2e) * (m^T m - a^T A^-1 a - t^T S^-1 t),
                     t = b - C^T (a/A)

Device plan (SPMD, 8 cores, FULL inputs replicated to every core):
  phase A: core p owns the 126-level window [126p, 126p+126) of the level-0
    axis. Per 128-row chunk (32 chunks = all N rows) ONE matmul with the
    stationary matrix [onehot0_window | 1 | m] (128 cols) against
    [onehot1 | 1 | m] (502 cols) accumulates, over all rows:
      rows 0..125: C slice | counts0 slice | a slice
      rows 126/127: the full G1 = [1|m]^T[Z1|1|m]  (identical on all cores)
    so counts0/a/G1/C all come from one PSUM bank. m, m^Tm, r^Tr are local.
  comm: ONE int8 AllGather (64KB/core): cols 0..500 = int8 C+counts slice,
    cols 504..507 = f32 a slice bit-cast (AllGather concatenates, bit-exact).
    A garbage warmup AllGather issued at t=0 pre-pays the NRT rendezvous
    barrier concurrently with phase A.
  phase C (redundant on all cores): S assembled tile-by-tile from the gathered
    buffer (the t-vector rides as rhs column 500); block LDL with 128-blocks;
    block inverses via Hotelling/Newton-Schulz; block logdets via Chebyshev
    trace of log accumulated on the PE.
"""

import math
import sys
import types

import numpy as np

import concourse.bass as bass
import concourse.bacc as bacc
import concourse.mybir as mybir
from concourse.bass import ds, ts
from concourse.bass_utils import run_bass_kernel_spmd
from concourse.masks import make_identity
from concourse.tile import TileContext


def _ensure_axon_hooks():
    """bass_utils imports antenv.axon_hooks when tracing; this image's antenv
    lacks it. Provide a shim (with the real ctypes NTFF hook when available)
    so trace=True/BASS_TRACE never crashes the kernel."""
    try:
        import antenv.axon_hooks  # noqa: F401
        return
    except ImportError:
        pass
    try:
        import trn_agent_boot.trn_boot as tb
        hook = tb._ntff_profile_via_ctypes("/opt/axon/libaxon_pjrt.so")
    except Exception:
        hook = None
    mod = types.ModuleType("antenv.axon_hooks")
    mod._hook = hook
    mod.get_axon_ntff_profile_hook = lambda: mod._hook

    def _set(h):
        mod._hook = h

    mod.set_axon_ntff_profile_hook = _set
    sys.modules["antenv.axon_hooks"] = mod
    try:
        import antenv
        antenv.axon_hooks = mod
    except ImportError:
        pass
    try:
        import concourse.bass_utils as bu
        _orig_upload = bu.upload_artifacts

        def _safe_upload(tmpdir):
            try:
                return _orig_upload(tmpdir)
            except Exception:
                return f"local:{tmpdir}"

        bu.upload_artifacts = _safe_upload
    except Exception:
        pass


_ensure_axon_hooks()

N = 4096
NCORES = 8
NCH = N // 128             # 32 row chunks, every core sees all of them
WIN = 126                  # level-0 window width per core (8*126=1008 >= 1000)
Q0 = 1000
Q1 = 500
FR = Q1 + 2                # rhs width: [Z1 | 1 | m]
AGW = 512                  # allgather row width (int8)
SP = 512                   # padded S size
NBLK = SP // 128           # 4
W3 = Q1 - 3 * 128          # 116: valid width of the last S block
LO, HI = 1.4, 18.0         # eigenvalue bounds for NS init + Chebyshev interval
NS_ITERS = 3
CHEB_DEG = 6
NCOEF = CHEB_DEG + 1
CLIP = 4.2648907939226017  # sqrt(2)*erfinv(1-2e-5)

F32 = mybir.dt.float32
BF16 = mybir.dt.bfloat16
I8 = mybir.dt.int8
I32 = mybir.dt.int32
U32 = mybir.dt.uint32
AX = mybir.AxisListType
OP = mybir.AluOpType
ACT = mybir.ActivationFunctionType


def cheb_coeffs(lo=LO, hi=HI, deg=CHEB_DEG):
    K = 4000
    th = (np.arange(K) + 0.5) * np.pi / K
    xk = np.cos(th)
    fk = np.log((hi - lo) / 2.0 * xk + (hi + lo) / 2.0)
    cs = np.array([2.0 / K * np.sum(fk * np.cos(j * th)) for j in range(deg + 1)])
    cs[0] *= 0.5
    return cs.astype(np.float32)


def _diag_fill(nc, tile_ap, value):
    nc.gpsimd.memset(tile_ap, 0.0)
    nc.gpsimd.affine_select(out=tile_ap, in_=tile_ap, compare_op=OP.not_equal,
                            fill=value, base=0, pattern=[[-1, 128]],
                            channel_multiplier=1)


def build_module(n_cores=NCORES):
    nc = bacc.Bacc(num_devices=n_cores)
    pk_d = nc.declare_dram_parameter("packed", [128, 4 * NCH], F32,
                                     isOutput=False)
    cst_d = nc.declare_dram_parameter("consts", [16], F32, isOutput=False)
    chb_d = nc.declare_dram_parameter("chebc", [2 * NCOEF], F32, isOutput=False)
    out_d = nc.declare_dram_parameter("out", [1, 1], F32, isOutput=True)

    agb_in = nc.dram_tensor("agb_in", [128 * AGW], I8)
    agb_out = nc.dram_tensor("agb_out", [n_cores * 128 * AGW], I8,
                             addr_space="Shared")
    warm_in = nc.dram_tensor("warm_in", [16], F32)
    warm_out = nc.dram_tensor("warm_out", [16 * n_cores], F32,
                              addr_space="Shared")

    with TileContext(nc) as tc, \
         tc.tile_pool(name="consts", bufs=1) as consts, \
         tc.tile_pool(name="work", bufs=1) as work:

        # warmup collective: pre-pays the one-time cross-core rendezvous
        # barrier concurrently with phase A. Data is garbage; result unused.
        if n_cores > 1:
            nc.gpsimd.collective_compute(
                "AllGather", OP.bypass,
                replica_groups=[list(range(n_cores))],
                ins=[warm_in[:]], outs=[warm_out[:]],
            )

        # ---- constants ----
        ident = consts.tile([128, 128], F32, tag="ident")
        make_identity(nc, ident)
        i2 = consts.tile([128, 128], F32, tag="i2")              # 2*I
        _diag_fill(nc, i2, 2.0)
        alphaI = consts.tile([128, 128], F32, tag="alphaI")      # NS init
        _diag_fill(nc, alphaI, 2.0 / (LO + HI))
        shiftI = consts.tile([128, 128], F32, tag="shiftI")      # Chebyshev shift
        _diag_fill(nc, shiftI, (HI + LO) / (HI - LO))
        ones512 = consts.tile([128, SP], F32, tag="ones512")
        nc.vector.memset(ones512, 1.0)
        identB16 = consts.tile([128, 128], BF16, tag="identB16")
        nc.vector.tensor_copy(identB16, ident)

        cst_row = consts.tile([1, 16], F32, tag="cst_row")
        nc.sync.dma_start(cst_row, cst_d[:].rearrange("(p x) -> p x", p=1))
        cst_row2 = consts.tile([1, 16], F32, tag="cst_row2")
        nc.vector.tensor_copy(cst_row2, cst_row)
        chb = consts.tile([1, 2 * NCOEF], F32, tag="chb")
        nc.sync.dma_start(chb, chb_d[:].rearrange("(p x) -> p x", p=1))
        chb2 = consts.tile([1, 2 * NCOEF], F32, tag="chb2")
        nc.vector.tensor_copy(chb2, chb)
        cst = consts.tile([128, 16], F32, tag="cst")
        chbB = consts.tile([128, 2 * NCOEF], F32, tag="chbB")
        with tc.tile_pool(name="setup_ps", bufs=2,
                          space=bass.MemorySpace.PSUM) as gps0:
            ps_b = gps0.tile([128, 16], F32, tag="gps0")
            nc.tensor.matmul(ps_b, ones512[0:1, 0:128], cst_row2,
                             start=True, stop=True)
            nc.vector.tensor_copy(cst, ps_b)
            ps_c = gps0.tile([128, 2 * NCOEF], F32, tag="gps0")
            nc.tensor.matmul(ps_c, ones512[0:1, 0:128], chb2,
                             start=True, stop=True)
            nc.vector.tensor_copy(chbB, ps_c)

        # iotas for the one-hot compares
        iota1i = work.tile([128, Q1], I32, tag="iota1i")
        nc.gpsimd.iota(iota1i, pattern=[[1, Q1]], base=0, channel_multiplier=0)
        iota1 = work.tile([128, Q1], F32, tag="iota1")
        nc.vector.tensor_copy(iota1, iota1i)
        iotaWi = work.tile([128, WIN], I32, tag="iotaWi")
        nc.gpsimd.iota(iotaWi, pattern=[[1, WIN]], base=0, channel_multiplier=0)
        iotaW = work.tile([128, WIN], F32, tag="iotaW")
        nc.vector.tensor_copy(iotaW, iotaWi)
        # shift by this core's window base (per-core const c[9])
        nc.vector.tensor_scalar(out=iotaW, in0=iotaW, scalar1=cst[:, 9:10],
                                scalar2=None, op0=OP.add)

        # ---- inputs -> m, sum r^2, sum m^2 ----
        packed = work.tile([128, 4 * NCH], F32, tag="packed")
        nc.sync.dma_start(packed, pk_d[:])
        yt = packed[:, 0:NCH]
        yp = packed[:, NCH:2 * NCH]
        idx0 = work.tile([128, NCH], F32, tag="idx0")
        nc.vector.tensor_copy(idx0, packed[:, 2 * NCH:3 * NCH].bitcast(I32))
        idx1 = work.tile([128, NCH], F32, tag="idx1")
        nc.vector.tensor_copy(idx1, packed[:, 3 * NCH:4 * NCH].bitcast(I32))
        resid = work.tile([128, NCH], F32, tag="resid")
        nc.vector.tensor_sub(resid, yt, yp)
        mvec = work.tile([128, NCH], F32, tag="mvec")
        nc.vector.tensor_scalar(out=mvec, in0=resid, scalar1=cst[:, 0:1],
                                scalar2=cst[:, 1:2], op0=OP.mult, op1=OP.min)
        nc.vector.tensor_scalar(out=mvec, in0=mvec, scalar1=cst[:, 8:9],
                                scalar2=None, op0=OP.max)
        scr_n = work.tile([128, NCH], F32, tag="scr_n")
        scal2 = work.tile([128, 2], F32, tag="scal2")
        nc.vector.tensor_mul(scr_n, resid, resid)
        nc.vector.tensor_reduce(scal2[:, 0:1], scr_n, AX.X, OP.add)
        nc.vector.tensor_mul(scr_n, mvec, mvec)
        nc.vector.tensor_reduce(scal2[:, 1:2], scr_n, AX.X, OP.add)
        scal2r = work.tile([1, 2], F32, tag="scal2r")
        with tc.tile_pool(name="sc_ps", bufs=1,
                          space=bass.MemorySpace.PSUM) as gpsc:
            ps_s = gpsc.tile([128, 2], F32, tag="gpsc")
            nc.tensor.matmul(ps_s[0:1, 0:2], ones512[:, 0:1], scal2,
                             start=True, stop=True)
            nc.vector.tensor_copy(scal2r, ps_s[0:1, 0:2])

        # ---- phase A: one matmul per 128-row chunk over all N rows ----
        G1 = work.tile([2, FR], F32, tag="G1")
        stg = work.tile([128, AGW], I8, tag="stg")
        with (
            tc.tile_pool(name="phA", bufs=3) as pha,
            tc.tile_pool(name="phA_ps", bufs=1, space=bass.MemorySpace.PSUM) as pps,
        ):
            psC = pps.tile([128, FR], F32, tag="psC")
            for c in range(NCH):
                cc = c % 3
                # cols 0/1 = [1|m] (G1 lands at psC rows 0/1: PSUM partition
                # reads must start at 0); cols 2..127 = level-window one-hot
                oh = pha.tile([128, 128], BF16, tag=f"oh_{cc}", name=f"oh_{cc}")
                nc.vector.memset(oh[:, 0:1], 1.0)
                nc.scalar.copy(oh[:, 1:2], mvec[:, c:c + 1])
                nc.vector.tensor_scalar(out=oh[:, 2:128], in0=iotaW,
                                        scalar1=idx0[:, c:c + 1],
                                        scalar2=None, op0=OP.is_equal)
                rh = pha.tile([128, FR], BF16, tag=f"rh_{cc}", name=f"rh_{cc}")
                nc.vector.tensor_scalar(out=rh[:, 0:Q1], in0=iota1,
                                        scalar1=idx1[:, c:c + 1],
                                        scalar2=None, op0=OP.is_equal)
                nc.vector.memset(rh[:, Q1:Q1 + 1], 1.0)
                nc.scalar.copy(rh[:, Q1 + 1:FR], mvec[:, c:c + 1])
                nc.tensor.matmul(psC, oh, rh, start=(c == 0),
                                 stop=(c == NCH - 1))
            # readout: G1 (rows 0/1), int8 staging for the AllGather
            nc.vector.tensor_copy(G1, psC[0:2, :])
            nc.vector.memset(stg[:, Q1 + 1:AGW], 0)
            nc.vector.tensor_copy(stg[:, 0:Q1 + 1], psC[:, 0:Q1 + 1])
            af32 = work.tile([128, 1], F32, tag="af32")
            nc.vector.tensor_copy(af32, psC[:, Q1 + 1:FR])
            nc.vector.tensor_copy(stg[:, 504:508], af32[:, 0:1].bitcast(I8))

        nc.sync.dma_start(
            agb_in[:].rearrange("(p f) -> p f", p=128), stg)
        if n_cores > 1:
            nc.gpsimd.collective_compute(
                "AllGather", OP.bypass,
                replica_groups=[list(range(n_cores))],
                ins=[agb_in[:]], outs=[agb_out[:]],
            )
        else:
            nc.sync.dma_start(agb_out[:], agb_in[:])

        # ---- phase C constants built while the collective is in flight ----
        iotaLi = work.tile([128, NCORES], I32, tag="iotaLi")
        nc.gpsimd.iota(iotaLi, pattern=[[0, NCORES]], base=0,
                       channel_multiplier=1)
        iotaL = work.tile([128, NCORES], F32, tag="iotaL")
        nc.vector.tensor_copy(iotaL, iotaLi)
        # per-tile valid rows: 2 <= row, row <= thr (level = 126t + row - 2;
        # rows 0/1 are G1 junk; tile 7 rows >= 120 are levels >= 1000 pad)
        thrM = work.tile([128, NCORES], F32, tag="thrM")
        nc.vector.memset(thrM, 127.5)
        nc.vector.memset(thrM[:, 7:8], float(Q0 - 7 * WIN) + 1.5)
        vmask = work.tile([128, NCORES], F32, tag="vmask")
        nc.vector.tensor_tensor(out=vmask, in0=iotaL, in1=thrM, op=OP.is_le)
        lowM = work.tile([128, NCORES], F32, tag="lowM")
        nc.vector.tensor_scalar(out=lowM, in0=iotaL, scalar1=1.5,
                                scalar2=None, op0=OP.is_gt)
        nc.vector.tensor_mul(vmask, vmask, lowM)
        padM = work.tile([128, NCORES], U32, tag="padM")
        nc.vector.tensor_scalar(out=padM, in0=vmask, scalar1=-1.0,
                                scalar2=1.0, op0=OP.mult, op1=OP.add)
        pm3 = work.tile([128, 1], U32, tag="pm3")
        nc.vector.tensor_scalar(out=pm3, in0=iotaL[:, 0:1],
                                scalar1=float(W3) - 0.5,
                                scalar2=None, op0=OP.is_gt)

        # G1-derived diagonal prep (runs inside the collective-wait window):
        # cbt[i] = [c1 | b] for S block i, dS[i] = D1 diag, dgblk[i] = dS*I,
        # plus the scaled diag tiles that let NS(0)/cheb(0) start straight
        # from the PSUM accumulators.
        alpha = 2.0 / (LO + HI)
        cheb_sc = 2.0 / (HI - LO)
        cbt = [work.tile([128, 2], F32, tag=f"cb{i}", name=f"cb{i}")
               for i in range(NBLK)]
        dSl = [work.tile([128, 1], F32, tag=f"dS{i}", name=f"dS{i}")
               for i in range(NBLK)]
        dgN = [work.tile([128, 128], BF16, tag=f"dg{i}", name=f"dg{i}")
               for i in range(NBLK)]
        with tc.tile_pool(name="g1_ps", bufs=2,
                          space=bass.MemorySpace.PSUM) as gps1:
            for i in range(NBLK):
                wi = 128 if i < NBLK - 1 else W3
                psT = gps1.tile([128, 2], F32, tag="pst")
                nc.tensor.transpose(psT[:wi, :], G1[0:2, ds(i * 128, wi)],
                                    ident[0:2, 0:2])
                nc.vector.memset(cbt[i], 0.0)
                nc.vector.tensor_copy(cbt[i][:wi, :], psT[:wi, :])
                nc.vector.tensor_scalar(out=dSl[i], in0=cbt[i][:, 0:1],
                                        scalar1=cst[:, 3:4],
                                        scalar2=None, op0=OP.add)
                if i == NBLK - 1:
                    nc.vector.copy_predicated(dSl[i], pm3, ones512[:, 0:1])
                    # block 3: pad-identity diag entries land at Srow cols
                    # >= 500, outside psS — keep the positive full-tile add
                    nc.vector.tensor_scalar_mul(dgN[i], ident, dSl[i])
                else:
                    # negated: folded into psS via a PE accumulation matmul
                    nc.vector.tensor_scalar(out=dSl[i], in0=dSl[i],
                                            scalar1=-1.0,
                                            scalar2=None, op0=OP.mult)
                    nc.vector.tensor_scalar_mul(dgN[i], ident, dSl[i])
        cI = []
        for j in range(NCOEF):
            cj = work.tile([128, 128], BF16, tag=f"cI{j}", name=f"cI{j}")
            nc.vector.tensor_scalar_mul(cj, ident, chbB[:, j:j + 1])
            cI.append(cj)

        # ---- S assembly from the gathered buffer, tile-pipelined ----
        Avec = work.tile([128, NCORES], F32, tag="Avec")
        aAll = work.tile([128, NCORES], F32, tag="aAll")
        Winv = work.tile([128, NCORES], F32, tag="Winv")
        Srow = [work.tile([128, SP], BF16, tag=f"Srow{i}", name=f"Srow{i}")
                for i in range(NBLK)]
        zvec = [work.tile([128, 1], F32, tag=f"z{i}", name=f"z{i}")
                for i in range(NBLK)]

        Binv = [work.tile([128, 128], F32, tag=f"Binv{k}", name=f"Binv{k}")
                for k in range(NBLK)]
        Wk = [work.tile([128, SP - (k + 1) * 128], BF16, tag=f"Wk{k}",
                        name=f"Wk{k}") for k in range(NBLK - 1)]
        Wk32 = [work.tile([128, SP - (k + 1) * 128], F32, tag=f"Wk32_{k}",
                          name=f"Wk32_{k}") for k in range(NBLK - 1)]
        trc = work.tile([128, NBLK], F32, tag="trc")
        qtt = work.tile([128, NBLK], F32, tag="qtt")

        with (
            tc.tile_pool(name="sasm", bufs=2) as sasm,
            tc.tile_pool(name="sasm_ps", bufs=1,
                         space=bass.MemorySpace.PSUM) as sps,
        ):
            ws = work.tile([128, NCORES], F32, tag="ws")
            xw = work.tile([128, NCORES], F32, tag="xw")
            psS = [sps.tile([128, Q1 + 1], F32, tag=f"psS{i}", name=f"psS{i}")
                   for i in range(NBLK)]
            for t in range(NCORES):
                tt = t % 3
                gt8 = sasm.tile([128, AGW], I8, tag=f"gt8_{tt}",
                                name=f"gt8_{tt}", bufs=3)
                # alternate DMA queues: each DMA costs ~600ns of queue issue
                deng = nc.sync if t % 2 == 0 else nc.gpsimd
                deng.dma_start(
                    gt8, agb_out[t * 128 * AGW:(t + 1) * 128 * AGW]
                    .rearrange("(p f) -> p f", p=128))
                # per-tile A/Winv/ws chain (t-local: tile pipeline starts
                # as soon as ITS gt8 lands, not after all of them)
                nc.vector.tensor_scalar(out=Avec[:, t:t + 1],
                                        in0=gt8[:, Q1:Q1 + 1],
                                        scalar1=cst[:, 2:3],
                                        scalar2=None, op0=OP.add)
                nc.vector.copy_predicated(Avec[:, t:t + 1], padM[:, t:t + 1],
                                          ones512[:, 0:1])
                nc.vector.reciprocal(Winv[:, t:t + 1], Avec[:, t:t + 1])
                nc.vector.tensor_mul(Winv[:, t:t + 1], Winv[:, t:t + 1],
                                     vmask[:, t:t + 1])
                nc.scalar.sqrt(ws[:, t:t + 1], Winv[:, t:t + 1])
                nc.vector.tensor_copy(aAll[:, t:t + 1],
                                      gt8[:, 504:508].bitcast(F32))
                nc.vector.tensor_mul(xw[:, t:t + 1], aAll[:, t:t + 1],
                                     ws[:, t:t + 1])
                # Ch = sqrt(1/A)-scaled C slice (bf16) — used on BOTH sides
                rhw = sasm.tile([128, Q1 + 1], BF16, tag=f"rhw_{tt}",
                                name=f"rhw_{tt}", bufs=3)
                nc.vector.tensor_scalar_mul(rhw[:, 0:Q1], gt8[:, 0:Q1],
                                            ws[:, t:t + 1])
                nc.vector.tensor_copy(rhw[:, Q1:Q1 + 1], xw[:, t:t + 1])
                for i in range(NBLK):
                    wi = 128 if i < NBLK - 1 else W3
                    nc.tensor.matmul(psS[i][:wi, :], rhw[:, ds(i * 128, wi)],
                                     rhw, start=(t == 0),
                                     stop=(t == NCORES - 1))
                if t == 0:
                    # fold the -D1 diagonal into the accumulation group
                    for i in range(NBLK - 1):
                        nc.tensor.matmul(psS[i][:128, ds(i * 128, 128)],
                                         dgN[i], identB16,
                                         start=False, stop=False)

            # logA and quad_a (off the matmul critical path)
            scr_t = work.tile([128, NCORES], F32, tag="scr_t")
            logA = work.tile([128, 1], F32, tag="logA")
            nc.scalar.activation(scr_t, Avec, ACT.Ln, accum_out=logA)
            qa = work.tile([128, 1], F32, tag="qa")
            nc.vector.tensor_mul(scr_t, xw, xw)
            nc.vector.tensor_reduce(qa, scr_t, AX.X, OP.add)

            # Srow / zvec assembly from psS (diag prep tiles precomputed)
            for i in range(NBLK):
                wi = 128 if i < NBLK - 1 else W3
                if i == NBLK - 1:
                    nc.vector.memset(Srow[i], 0.0)
                else:
                    nc.vector.memset(Srow[i][:, Q1:SP], 0.0)
                nc.vector.tensor_scalar_mul(Srow[i][:wi, 0:Q1],
                                            psS[i][:wi, 0:Q1], -1.0)
                if i == NBLK - 1:
                    nc.vector.tensor_add(Srow[i][:, ts(i, 128)],
                                         Srow[i][:, ts(i, 128)], dgN[i])
                nc.vector.memset(zvec[i], 0.0)
                nc.vector.tensor_sub(zvec[i][:wi, :], cbt[i][:wi, 1:2],
                                     psS[i][:wi, Q1:Q1 + 1])

        with (
            tc.tile_pool(name="ldl", bufs=4) as ldl,
            tc.tile_pool(name="ldl_ps", bufs=4,
                         space=bass.MemorySpace.PSUM) as lps,
            tc.tile_pool(name="rps_pool", bufs=1,
                         space=bass.MemorySpace.PSUM) as rpsp,
        ):
            # ---- block LDL (Hotelling/NS inverses) with the Chebyshev
            # trace recurrences emission-interleaved into the NS bubbles.
            # Traces via tensor_tensor_reduce on DVE (no PE accumulation).
            cheb = {}          # k -> [b2, tprev, tcur, next_j]
            pending = []
            Rps = []

            def cheb_start(k):
                bh = ldl.tile([128, 128], BF16, tag=f"bh{k}", name=f"bh{k}")
                Bk = Srow[k][:, ts(k, 128)]
                nc.vector.tensor_scalar_mul(bh, Bk, cheb_sc)
                nc.vector.tensor_sub(bh, bh, shiftI)
                b2 = ldl.tile([128, 128], BF16, tag=f"b2{k}", name=f"b2{k}")
                nc.vector.tensor_scalar_mul(b2, bh, 2.0)
                R = rpsp.tile([128, 128], F32, tag=f"Rps{k}", name=f"Rps{k}")
                Rps.append(R)
                nc.tensor.matmul(R, cI[1], bh, start=True, stop=False)
                cheb[k] = [b2, identB16, bh, 2]
                pending.append(k)

            def cheb_round():
                if not pending:
                    return
                k = pending.pop(0)
                b2, tprev, tcur, j = cheb[k]
                psc = lps.tile([128, 128], F32, tag="lps")
                nc.tensor.matmul(psc, b2, tcur, start=True, stop=True)
                tnext = ldl.tile([128, 128], BF16, tag=f"chT{k}",
                                 name=f"chT{k}_{j}", bufs=3)
                nc.vector.tensor_sub(tnext, psc, tprev)
                nc.tensor.matmul(Rps[k], cI[j], tnext, start=False,
                                 stop=(j == CHEB_DEG))
                cheb[k] = [b2, tcur, tnext, j + 1]
                if j + 1 <= CHEB_DEG:
                    pending.append(k)

            for k in range(NBLK):
                # D_k is final here (panel k-1 updates already applied)
                cheb_start(k)
                trail = SP - (k + 1) * 128 if k < NBLK - 1 else 0
                Y = ldl.tile([128, 128], BF16, tag="nsY")
                nc.vector.tensor_scalar_mul(Y, Srow[k][:, ts(k, 128)], alpha)
                Z = ldl.tile([128, 128], BF16, tag="nsZ")
                nc.vector.tensor_sub(Z, i2, Y)
                X = ldl.tile([128, 128], BF16, tag="nsX")
                nc.vector.tensor_copy(X, alphaI)
                psX = None
                for it in range(NS_ITERS):
                    last = it == NS_ITERS - 1
                    if not last:
                        psY = lps.tile([128, 128], F32, tag="lps")
                        nc.tensor.matmul(psY, Y, Z, start=True, stop=True)
                    psX = lps.tile([128, 128], F32, tag="lps")
                    nc.tensor.matmul(psX, X, Z, start=True, stop=True)
                    cheb_round()
                    X = ldl.tile([128, 128], BF16, tag="nsX")
                    nc.vector.tensor_copy(X, psX)
                    if not last:
                        Z = ldl.tile([128, 128], BF16, tag="nsZ")
                        nc.vector.tensor_sub(Z, i2, psY)
                        Y = ldl.tile([128, 128], BF16, tag="nsY")
                        nc.scalar.copy(Y, psY)
                nc.scalar.copy(Binv[k], psX)
                if trail:
                    psW = lps.tile([128, 384], F32, tag="lps")
                    nc.tensor.matmul(psW[:, :trail], X,
                                     Srow[k][:, (k + 1) * 128:SP],
                                     start=True, stop=True)
                    nc.vector.tensor_copy(Wk[k], psW[:, :trail])
                    nc.scalar.copy(Wk32[k], psW[:, :trail])
                    for i in range(k + 1, NBLK):
                        psu = lps.tile([128, 384], F32, tag="lps")
                        nc.tensor.matmul(psu[:, :trail], Srow[k][:, ts(i, 128)],
                                         Wk[k], start=True, stop=True)
                        nc.vector.tensor_sub(Srow[i][:, (k + 1) * 128:SP],
                                             Srow[i][:, (k + 1) * 128:SP],
                                             psu[:, :trail])
                    cheb_round()
                    cheb_round()
            while pending:
                cheb_round()
            for k in range(NBLK):
                Rsb = ldl.tile([128, 128], F32, tag="Rsb")
                nc.vector.tensor_mul(Rsb, Rps[k], ident)   # diagonal only
                nc.vector.tensor_reduce(trc[:, k:k + 1], Rsb, AX.X, OP.add)

            # forward substitution: z_i -= (Wk[k] block i)^T z_k
            for k in range(NBLK - 1):
                for i in range(k + 1, NBLK):
                    psz = lps.tile([128, 1], F32, tag="lps")
                    off = (i - k - 1) * 128
                    nc.tensor.matmul(psz, Wk32[k][:, ds(off, 128)], zvec[k],
                                     start=True, stop=True)
                    nc.vector.tensor_sub(zvec[i], zvec[i], psz)
            # quad_t = sum_k z_k^T Binv_k z_k
            for k in range(NBLK):
                psq = lps.tile([128, 1], F32, tag="lps")
                nc.tensor.matmul(psq, Binv[k], zvec[k], start=True, stop=True)
                uk = ldl.tile([128, 1], F32, tag="uk")
                nc.vector.tensor_copy(uk, psq)
                nc.vector.tensor_mul(qtt[:, k:k + 1], zvec[k], uk)

        # ---- final scalar assembly ----
        qtr = work.tile([128, 1], F32, tag="qtr")
        nc.vector.tensor_reduce(qtr, qtt, AX.X, OP.add)
        smalls_c = work.tile([128, 3 + NBLK], F32, tag="smalls_c")
        nc.vector.tensor_copy(smalls_c[:, 0:1], logA)
        nc.vector.tensor_copy(smalls_c[:, 1:2], qa)
        nc.vector.tensor_copy(smalls_c[:, 2:3], qtr)
        nc.vector.tensor_copy(smalls_c[:, 3:3 + NBLK], trc)
        smalls = work.tile([1, 3 + NBLK], F32, tag="smalls")
        ldS = work.tile([1, 1], F32, tag="ldS")
        with tc.tile_pool(name="fin_ps", bufs=1,
                          space=bass.MemorySpace.PSUM) as gps2:
            ps_sm = gps2.tile([128, 3 + NBLK], F32, tag="gps2")
            nc.tensor.matmul(ps_sm[0:1, :], ones512[:, 0:1], smalls_c,
                             start=True, stop=True)
            nc.vector.tensor_copy(smalls, ps_sm[0:1, :])
        nc.vector.tensor_reduce(ldS, smalls[:, 3:3 + NBLK], AX.X, OP.add)

        fin = work.tile([1, 8], F32, tag="fin")
        mtm = scal2r[:, 1:2]
        r2g = scal2r[:, 0:1]
        # quadK = quad_a + quad_t
        nc.vector.tensor_add(fin[:, 0:1], smalls[:, 1:2], smalls[:, 2:3])
        # mVinvm = (sig2/sig2e) * (mtm - quadK)
        nc.vector.tensor_sub(fin[:, 1:2], mtm, fin[:, 0:1])
        nc.vector.tensor_scalar_mul(fin[:, 1:2], fin[:, 1:2], cst[0:1, 6:7])
        # logdetV = const1 + sum log A + logdet S
        nc.vector.tensor_add(fin[:, 2:3], smalls[:, 0:1], ldS)
        nc.vector.tensor_scalar(out=fin[:, 2:3], in0=fin[:, 2:3],
                                scalar1=cst[0:1, 4:5], scalar2=None, op0=OP.add)
        # sum_log_pdf = const2 - sum_r2/(2 sig2)
        nc.vector.tensor_scalar(out=fin[:, 3:4], in0=r2g, scalar1=cst[0:1, 7:8],
                                scalar2=cst[0:1, 5:6], op0=OP.mult, op1=OP.add)
        # total = 0.5*(logdetV + mVinvm - mtm + sum_log_pdf)
        nc.vector.tensor_add(fin[:, 4:5], fin[:, 2:3], fin[:, 1:2])
        nc.vector.tensor_sub(fin[:, 4:5], fin[:, 4:5], mtm)
        nc.vector.tensor_add(fin[:, 4:5], fin[:, 4:5], fin[:, 3:4])
        nc.vector.tensor_scalar_mul(fin[:, 4:5], fin[:, 4:5], 0.5)

        nc.sync.dma_start(out_d[:], fin[:, 4:5])

    nc.finalize()
    return nc


def host_consts(sig2e, sig2bs, core_id):
    s0, s1 = float(sig2bs[0]), float(sig2bs[1])
    sig2e = float(sig2e)
    sig2 = sig2e + s0 + s1
    c = np.zeros(16, np.float32)
    c[0] = 1.0 / math.sqrt(sig2)
    c[1] = CLIP
    c[2] = sig2e / s0
    c[3] = sig2e / s1
    # the j=0 Chebyshev trace term (c_0 * 128 per block) folds in here
    c[4] = ((N - Q0 - Q1) * math.log(sig2e) + Q0 * math.log(s0)
            + Q1 * math.log(s1) - N * math.log(sig2)
            + NBLK * 128 * float(cheb_coeffs()[0]))
    c[5] = -0.5 * N * math.log(2.0 * math.pi * sig2)
    c[6] = sig2 / sig2e
    c[7] = -1.0 / (2.0 * sig2)
    c[8] = -CLIP
    c[9] = float(WIN * core_id)
    return c


_CACHE = {}


def _get_module(n_cores=NCORES):
    if n_cores not in _CACHE:
        _CACHE[n_cores] = build_module(n_cores)
    return _CACHE[n_cores]


def make_in_maps(inputs, n_cores=NCORES):
    y_true = np.ascontiguousarray(np.asarray(inputs["y_true"], np.float32).reshape(N, 1))
    y_pred = np.ascontiguousarray(np.asarray(inputs["y_pred"], np.float32).reshape(N, 1))
    zi0 = np.ascontiguousarray(np.asarray(inputs["Z_idx0"]).astype(np.int32).reshape(N))
    zi1 = np.ascontiguousarray(np.asarray(inputs["Z_idx1"]).astype(np.int32).reshape(N))
    cs = cheb_coeffs().astype(np.float32)
    chebc = np.concatenate([cs, np.zeros(NCOEF, np.float32)])
    pk = np.concatenate([
        y_true.reshape(NCH, 128).T,
        y_pred.reshape(NCH, 128).T,
        zi0.reshape(NCH, 128).T.view(np.float32),
        zi1.reshape(NCH, 128).T.view(np.float32),
    ], axis=1)
    pk = np.ascontiguousarray(pk)
    maps = []
    for i in range(n_cores):
        c = host_consts(np.asarray(inputs["sig2e"]),
                        np.asarray(inputs["sig2bs"], np.float64), i)
        maps.append({"packed": pk, "consts": c, "chebc": chebc})
    return maps


def kernel(**inputs):
    nc = _get_module(NCORES)
    maps = make_in_maps(inputs, NCORES)
    res = run_bass_kernel_spmd(nc, maps, list(range(NCORES)))
    out = np.asarray(res.results[0]["out"], np.float32).reshape(1, 1)
    return out
